# revision 38
# baseline (speedup 1.0000x reference)
"""Trainium2 kernel: per-pixel channel-mixing attention via temperature
interpolation (sigma-interp), v8.

Math per pixel: out_i = sum_j sigma_i(k_j) q_j where sigma(t) = softmax(t*v)
over channels. sigma(t*v) is interpolated in the temperature t at M=11
per-pixel-scaled Chebyshev nodes t_m = Tk*u_m (Tk = max|k| per pixel):

    out_i = sum_m exp(u_m * Tk*v_i) * S_m,   S_m = R_m / G_m
    G_m   = sum_i exp(u_m * Tk*v_i)
    R_m   = sum_r Lc[r,m] * That_r,  That_r = sum_j T_r(k_j/Tk) q_j

M=11 includes the center node u=0 whose grid is identically 1: no exp pass,
no eval multiply; its S broadcast feeds the output sum directly.

Host sorts pixels by A = max|v|*max|k|; the hardest 128 per core go through
an exact pair-grid tile.

Performance structure (v8):
  - R produced directly by Lc-weighted reduction stationaries.
  - chebyshev chain parity-split over z = 2*khat^2-1 (half the serial depth).
  - warmup/filler matmuls run in the unused partition rows of the reduce
    psum tile (same banks, disjoint partitions) to hold the PE clock at
    2.4 GHz through analysis.
  - eval: S broadcast (PE) -> fp16 SBUF copy (ACT) -> 2x DVE multiply ->
    pairwise fp16 DVE add-tree.  No identity-accumulate matmuls at all:
    the eval PE work is just the broadcasts, deeply pipelined through a
    4-buffer PSUM pool (the accumulator banks are freed by the tree).
  - exact-tile work is interleaved into engine slack around the S phase.
  - no Ln activations (reciprocal_approx_fast): one exp table load.
"""

import sys

sys.path.insert(0, "/opt/trn_rl_repo")

from contextlib import ExitStack

import ml_dtypes
import numpy as np

import concourse.bacc as bacc
import concourse.bass as bass
import concourse.tile as tile
from concourse import mybir
from concourse.bass_utils import run_bass_kernel_spmd

B, C, H, W = 2, 64, 128, 128
N_CORES = 8
NPIX = B * H * W            # 32768
M = 11                      # interp nodes (odd: center node u=0 is free)
MC = M // 2
NEX_CORE = 128              # exact pixels per core
NEZ_CORE = NPIX // N_CORES - NEX_CORE   # 3968 interp pixels per core
FD = NEZ_CORE // 2          # 1984 pixels per g-half
HFD = FD // 2               # 992 pixels per column-half
R2 = 2 * M
N_WARM = 24                 # PE warmup matmuls
N_JUNK_ANA = 4              # filler matmuls per analysis m-step
# (half, m) slots whose S stays in PSUM (DVE 1x mul, no ACT copy)
PSUM_MUL_SLOTS = {(0, 2), (1, 2), (0, 8), (1, 8)}

FP32 = mybir.dt.float32
FP16 = mybir.dt.float16
BF16 = mybir.dt.bfloat16
EXP = mybir.ActivationFunctionType.Exp

U_NODES = np.cos(np.pi * np.arange(M) / (M - 1))


def _lc_matrix():
    u = U_NODES
    Tn = np.cos(np.arange(M)[:, None] * np.arccos(np.clip(u, -1, 1))[None, :])
    return np.linalg.inv(Tn.T)


def build_kernel():
    nc = bacc.Bacc(
        "TRN2",
        target_bir_lowering=False,
        debug=False,
        enable_asserts=False,
        num_devices=N_CORES,
    )
    vp = nc.dram_tensor("vp", [128, FD], FP16, kind="ExternalInput").ap()
    k2t = nc.dram_tensor("k2t", [128, FD], FP16, kind="ExternalInput").ap()
    qt = nc.dram_tensor("qt", [128, FD], FP16, kind="ExternalInput").ap()
    u1t = nc.dram_tensor("u1t", [128, FD], FP16, kind="ExternalInput").ap()
    statR = nc.dram_tensor("statR", [128, M, R2], FP16, kind="ExternalInput").ap()
    statL = nc.dram_tensor("statL", [128, M, R2], FP16, kind="ExternalInput").ap()
    statB = nc.dram_tensor("statB", [R2, M, 128], FP16, kind="ExternalInput").ap()
    vE = nc.dram_tensor("vE", [128, C], FP32, kind="ExternalInput").ap()
    kE = nc.dram_tensor("kE", [128, C], FP32, kind="ExternalInput").ap()
    qE = nc.dram_tensor("qE", [128, C], FP32, kind="ExternalInput").ap()

    outm = nc.dram_tensor("outm", [128, FD], FP16, kind="ExternalOutput").ap()
    oute = nc.dram_tensor("oute", [128, C], FP32, kind="ExternalOutput").ap()

    CH = [0, 512, HFD]

    with tile.TileContext(nc) as tc, ExitStack() as ctx:
        sb = ctx.enter_context(tc.tile_pool(name="sb", bufs=1))
        sbw = ctx.enter_context(tc.tile_pool(name="sbw", bufs=1))
        sbp = ctx.enter_context(tc.tile_pool(name="sbp", bufs=2))

        wu = sbw.tile([128, 512], FP16, tag="wu")
        nc.vector.memset(wu, 1.0)

        # dummy activation: pull the exp table load off the critical path
        dum = sbw.tile([1, 16], FP32, tag="dum")
        nc.scalar.activation(out=dum, in_=wu[0:1, 0:16], func=EXP)

        # ---- input DMAs ----
        v_t = sb.tile([128, FD], FP16)
        k2_t = sb.tile([128, FD], FP16)
        q_t = sb.tile([128, FD], FP16)
        u1_t = sb.tile([128, FD], FP16)
        sR = sb.tile([128, M, R2], FP16)
        sL = sb.tile([128, M, R2], FP16)
        sB_t = sb.tile([R2, M, 128], FP16)
        nc.sync.dma_start(out=v_t[:, :FD // 2], in_=vp[:, :FD // 2])
        nc.sync.dma_start(out=k2_t, in_=k2t)
        nc.sync.dma_start(out=u1_t, in_=u1t)
        nc.sync.dma_start(out=q_t, in_=qt)
        nc.sync.dma_start(out=v_t[:, FD // 2:], in_=vp[:, FD // 2:])
        nc.gpsimd.dma_start(out=sR, in_=statR)
        nc.gpsimd.dma_start(out=sL, in_=statL)
        nc.gpsimd.dma_start(out=sB_t, in_=statB)

        ones_t = sbw.tile([128, FD], FP16, tag="ones")
        nc.vector.memset(ones_t, 1.0)

        # exact-tile small inputs (early: cheap, off the critical path)
        vE_t = sb.tile([128, C], FP32)
        nc.sync.dma_start(out=vE_t, in_=vE)
        kE_t = sb.tile([128, C], FP32)
        nc.sync.dma_start(out=kE_t, in_=kE)
        qE_t = sb.tile([128, C], FP32)
        nc.sync.dma_start(out=qE_t, in_=qE)
        v2 = sb.tile([128, C, 2], FP16)
        nc.scalar.copy(v2, vE_t[:, :, None].broadcast_to([128, C, 2]))
        kE16 = sb.tile([128, C], FP16)
        nc.scalar.copy(kE16, kE_t)

        # ---- X grids (ACT, fp16), per column-half for early start ----
        Xs = {}
        for m in range(M):
            if m == MC:
                Xs[m] = ones_t
                continue
            xm = sbw.tile([128, FD], FP16, tag=f"x{m}")
            nc.scalar.activation(out=xm[:, :FD // 2], in_=v_t[:, :FD // 2],
                                 func=EXP, scale=float(U_NODES[m]))
            nc.scalar.activation(out=xm[:, FD // 2:], in_=v_t[:, FD // 2:],
                                 func=EXP, scale=float(U_NODES[m]))
            Xs[m] = xm

        # ---- chebyshev chain, parity split over zz = 2z = k2^2 - 2 ----
        zz = sbw.tile([128, FD], FP16, tag="zz")
        nc.vector.tensor_mul(zz, k2_t, k2_t)
        nc.vector.tensor_scalar_add(zz, zz, -2.0)
        Us = {0: q_t, 1: u1_t}
        e1 = sbw.tile([128, FD], FP16, tag="e1")
        nc.vector.scalar_tensor_tensor(
            out=e1, in0=zz, scalar=0.5, in1=q_t,
            op0=mybir.AluOpType.mult, op1=mybir.AluOpType.mult)
        Us[2] = e1
        o1 = sbw.tile([128, FD], FP16, tag="o1")
        nc.vector.scalar_tensor_tensor(
            out=o1, in0=zz, scalar=-1.0, in1=u1_t,
            op0=mybir.AluOpType.add, op1=mybir.AluOpType.mult)
        Us[3] = o1
        for par in (0, 1):
            prev, cur = Us[0 + par], Us[2 + par]
            for r in range(4 + par, M, 2):
                tmp = sbp.tile([128, FD], FP16, tag=f"tmp{par}")
                nc.vector.tensor_mul(tmp, zz, cur)
                nxt = sbw.tile([128, FD], FP16, tag=f"u{r}")
                nc.vector.tensor_sub(nxt, tmp, prev)
                Us[r] = nxt
                prev, cur = cur, nxt

        # exact pair grid (DVE work early; ACT exp deferred to S phase)
        P_t = sb.tile([128, C, C], FP16)
        k_op = bass.AP(
            tensor=kE16.tensor, offset=kE16.offset,
            ap=[kE16.ap[0], [0, C], [2, C // 2], [1, 2]],
        )
        v_op = bass.AP(
            tensor=v2.tensor, offset=v2.offset,
            ap=[v2.ap[0], [2, C], [0, C // 2], [1, 2]],
        )
        nc.vector.tensor_mul(
            P_t.rearrange("p i (jh jp) -> p i jh jp", jp=2), k_op, v_op)

        # ---- analysis reductions: R rows 0:22 (Lc-weighted), G rows 32:54
        # warmup + filler matmuls live in rows 64:128 of the same tiles.
        with tc.tile_pool(name="red", bufs=2, space="PSUM") as redp:
            red_ts = [redp.tile([128, HFD], FP32, tag="red", name=f"red{h}")
                      for h in range(2)]
            _junk_i = [0]

            def junk_mm(n):
                for _ in range(n):
                    t = red_ts[_junk_i[0] % 2]
                    _junk_i[0] += 1
                    nc.tensor.matmul(t[64:128, 0:496], wu[:, 0:64],
                                     wu[:, 0:496], start=True, stop=True,
                                     skip_group_check=True)

            junk_mm(N_WARM)
            for m in range(M):
                for h in range(2):
                    hs = slice(h * HFD, (h + 1) * HFD)
                    r_ps = red_ts[h][0:R2, :]
                    g_ps = red_ts[h][32:32 + R2, :]
                    for a, b in zip(CH[:-1], CH[1:]):
                        nc.tensor.matmul(r_ps[:, a:b], sL[:, m, :],
                                         Us[m][:, hs][:, a:b],
                                         start=(m == 0), stop=(m == M - 1))
                        nc.tensor.matmul(g_ps[:, a:b], sR[:, m, :],
                                         Xs[m][:, hs][:, a:b],
                                         start=(m == 0), stop=(m == M - 1))
                junk_mm(N_JUNK_ANA)

            # ---- S = R / G per half (highest scheduler priority: the
            # whole eval stream is gated on these few small ops) ----
            s_halves = []
            for h in range(2):
                r_ps = red_ts[h][0:R2, :]
                g_ps = red_ts[h][32:32 + R2, :]
                with tc.high_priority():
                    gsb = sbw.tile([R2, HFD], FP32, tag=f"gsb{h}")
                    nc.scalar.copy(gsb, g_ps)
                    ginv = sbw.tile([R2, HFD], FP32, tag=f"ginv{h}")
                    nc.vector.reciprocal_approx_fast(out=ginv, in_=gsb)
                    s_th = sbw.tile([R2, HFD], FP16, tag=f"s{h}")
                    nc.vector.tensor_mul(s_th, ginv, r_ps)
                s_halves.append(s_th)
                junk_mm(11)

        # exact-tile exp: scalar-engine slack right after the X stream
        E_t = sb.tile([128, C, C], BF16)
        for eb in range(4):
            nc.scalar.activation(out=E_t[:, eb * 16:(eb + 1) * 16, :],
                                 in_=P_t[:, eb * 16:(eb + 1) * 16, :],
                                 func=EXP)

        # ---- eval: out = sum_m X_m * bcast(S_m), fp16 DVE add-tree ----
        with tc.tile_pool(name="evp", bufs=4, space="PSUM") as evp:
            pend = {0: [], 1: []}   # binary-counter tree: (level, tile)

            def tree_push(half, t, level=0):
                pend[half].append((level, t))
                while (len(pend[half]) >= 2
                       and pend[half][-1][0] == pend[half][-2][0]):
                    l1, a2 = pend[half].pop()
                    _, b2 = pend[half].pop()
                    nc.vector.tensor_add(a2, a2, b2)
                    pend[half].append((l1 + 1, a2))

            def eval_slot(m, half):
                s_h = s_halves[half]
                s_b = evp.tile([128, HFD], FP32, tag="sbps",
                               name=f"sb{half}_{m}")
                for a, b in zip(CH[:-1], CH[1:]):
                    nc.tensor.matmul(s_b[:, a:b], sB_t[:, m, :],
                                     s_h[:, a:b], start=True, stop=True)
                if m == MC:
                    s_bs = sbp.tile([128, HFD], FP16, tag="sbs", bufs=6)
                    nc.scalar.copy(s_bs, s_b)
                    tree_push(half, s_bs)
                    return
                prod = sbp.tile([128, HFD], FP16, tag="prod", bufs=8)
                if (half, m) in PSUM_MUL_SLOTS:
                    nc.vector.tensor_mul(
                        prod, Xs[m][:, half * HFD:(half + 1) * HFD], s_b)
                else:
                    s_bs = sbp.tile([128, HFD], FP16, tag="sbs", bufs=6)
                    nc.scalar.copy(s_bs, s_b)
                    nc.vector.tensor_mul(
                        prod, Xs[m][:, half * HFD:(half + 1) * HFD], s_bs)
                tree_push(half, prod)

            # order: MC first (cheap), then the rest; halves interleaved
            order = [MC] + [m for m in range(M) if m != MC]
            for i, m in enumerate(order):
                for half in range(2):
                    eval_slot(m, half)
                if i == 1:
                    G1 = sb.tile([128, C // 4, C], BF16)
                    G2 = sb.tile([128, C // 4, C], BF16)
                    nc.vector.tensor_add(G1, E_t[:, : C // 4, :],
                                         E_t[:, C // 4: C // 2, :])
                    nc.vector.tensor_add(G2, E_t[:, C // 2: 3 * C // 4, :],
                                         E_t[:, 3 * C // 4:, :])
                    nc.vector.tensor_add(G1, G1, G2)
                if i == 3:
                    nc.vector.tensor_add(G1[:, : C // 8, :], G1[:, : C // 8, :],
                                         G1[:, C // 8: C // 4, :])
                    nc.vector.tensor_add(G1[:, : C // 16, :],
                                         G1[:, : C // 16, :],
                                         G1[:, C // 16: C // 8, :])
                    d_t = sb.tile([128, C], FP32)
                    nc.vector.tensor_reduce(
                        out=d_t, in_=G1[:, : C // 16, :].transpose([0, 2, 1]),
                        axis=mybir.AxisListType.X, op=mybir.AluOpType.add,
                    )
                    r_t = sb.tile([128, C], FP32)
                    nc.vector.reciprocal_approx_fast(out=r_t, in_=d_t)
                    w16 = sb.tile([128, C], BF16)
                    nc.vector.tensor_mul(w16, qE_t, r_t)
                if i == 4:
                    Q4 = C // 4
                    F1 = sb.tile([128, C, Q4], BF16)
                    F2 = sb.tile([128, C, Q4], BF16)
                    F3 = sb.tile([128, C, Q4], BF16)
                    F4 = sb.tile([128, C, Q4], BF16)
                    for fi, Fq in enumerate((F1, F2, F3, F4)):
                        nc.vector.tensor_mul(
                            Fq, E_t[:, :, fi * Q4: (fi + 1) * Q4],
                            w16[:, None, fi * Q4: (fi + 1) * Q4]
                            .broadcast_to([128, C, Q4]),
                        )

                if i == 8:
                    nc.vector.tensor_add(F1, F1, F2)
                    nc.vector.tensor_add(F3, F3, F4)
                    nc.vector.tensor_add(F1, F1, F3)
                    nc.vector.tensor_add(F1[:, :, : Q4 // 2],
                                         F1[:, :, : Q4 // 2],
                                         F1[:, :, Q4 // 2:])
                    nc.vector.tensor_add(F1[:, :, : Q4 // 4],
                                         F1[:, :, : Q4 // 4],
                                         F1[:, :, Q4 // 4: Q4 // 2])
                    oE = sb.tile([128, C], FP32)
                    nc.vector.tensor_reduce(
                        out=oE, in_=F1[:, :, : Q4 // 4],
                        axis=mybir.AxisListType.X, op=mybir.AluOpType.add,
                    )
                    nc.sync.dma_start(out=oute, in_=oE)

            # drain each half's tree; final merge per column-chunk with
            # the output DMA of each chunk issued immediately
            for half in range(2):
                while len(pend[half]) > 2:
                    _, a2 = pend[half].pop()
                    l2, b2 = pend[half].pop()
                    nc.vector.tensor_add(a2, a2, b2)
                    pend[half].append((l2 + 1, a2))
                if len(pend[half]) == 2:
                    _, rootA = pend[half].pop()
                    _, rootB = pend[half].pop()
                    for qd in range(2):
                        ss = slice(qd * 496, (qd + 1) * 496)
                        gs = slice(half * HFD + qd * 496,
                                   half * HFD + (qd + 1) * 496)
                        nc.vector.tensor_add(rootA[:, ss], rootA[:, ss],
                                             rootB[:, ss])
                        nc.sync.dma_start(out=outm[:, gs], in_=rootA[:, ss])
                else:
                    nc.sync.dma_start(
                        out=outm[:, half * HFD:(half + 1) * HFD],
                        in_=pend[half][0][1])

    nc.compile()
    return nc


_NC_CACHE = None


def _get_nc():
    global _NC_CACHE
    if _NC_CACHE is None:
        _NC_CACHE = build_kernel()
    return _NC_CACHE


def _prep(x, y, z):
    """Host prep: sort by difficulty, shard, scale. Returns in_maps + meta."""
    q = np.ascontiguousarray(np.transpose(np.asarray(x), (0, 2, 3, 1))).reshape(-1, C)
    k = np.ascontiguousarray(np.transpose(np.asarray(y), (0, 2, 3, 1))).reshape(-1, C)
    v = np.ascontiguousarray(np.transpose(np.asarray(z), (0, 2, 3, 1))).reshape(-1, C)
    Tk = np.abs(k).max(axis=1)
    A = Tk * np.abs(v).max(axis=1)
    order = np.argsort(A, kind="stable")
    easy = order[: NEZ_CORE * N_CORES]
    hard = order[NEZ_CORE * N_CORES:]

    Lc = _lc_matrix()
    statR = np.zeros((128, M, R2), np.float32)
    for m in range(M):
        for g in range(2):
            statR[g * 64:(g + 1) * 64, m, 2 * m + g] = 1
    statL = np.zeros((128, M, R2), np.float32)
    for r in range(M):
        for m in range(M):
            for g in range(2):
                statL[g * 64:(g + 1) * 64, r, 2 * m + g] = Lc[r, m]
    statB = np.zeros((R2, M, 128), np.float32)
    for m in range(M):
        for g in range(2):
            statB[2 * m + g, m, g * 64:(g + 1) * 64] = 1

    in_maps = []
    meta = []
    for c in range(N_CORES):
        ez = easy[c::N_CORES]
        hd = hard[c::N_CORES]
        kh = k[ez] / Tk[ez, None]
        vp_c = (Tk[ez, None] * v[ez]).astype(np.float16)
        k2_c = (2.0 * kh).astype(np.float16)
        q_c = q[ez].astype(np.float16)
        u1_c = (kh * q[ez]).astype(np.float16)

        def cmaj(a2d, dt):
            h0 = a2d[:FD].T
            h1 = a2d[FD:].T
            return np.ascontiguousarray(np.concatenate([h0, h1], axis=0)).astype(dt)

        in_maps.append({
            "vp": cmaj(vp_c, np.float16),
            "k2t": cmaj(k2_c, np.float16),
            "qt": cmaj(q_c, np.float16),
            "u1t": cmaj(u1_c, np.float16),
            "statR": statR.astype(np.float16),
            "statL": statL.astype(np.float16),
            "statB": statB.astype(np.float16),
            "vE": v[hd].astype(np.float32),
            "kE": k[hd].astype(np.float32),
            "qE": q[hd].astype(np.float32),
        })
        meta.append((ez, hd))
    return in_maps, meta


def kernel(x, y, z):
    nc = _get_nc()
    in_maps, meta = _prep(x, y, z)
    res = run_bass_kernel_spmd(nc, in_maps, core_ids=list(range(N_CORES)))
    out = np.empty((NPIX, C), np.float32)
    for c in range(N_CORES):
        ez, hd = meta[c]
        om = res.results[c]["outm"].astype(np.float32)
        out[ez[:FD]] = om[:64].T
        out[ez[FD:]] = om[64:].T
        out[hd] = res.results[c]["oute"]
    return np.ascontiguousarray(
        np.transpose(out.reshape(B, H, W, C), (0, 3, 1, 2))
    ).astype(np.float32)


# revision 39
# speedup vs baseline: 1.0245x; 1.0245x over previous
"""Trainium2 kernel: per-pixel channel-mixing attention via temperature
interpolation (sigma-interp), v8.

Math per pixel: out_i = sum_j sigma_i(k_j) q_j where sigma(t) = softmax(t*v)
over channels. sigma(t*v) is interpolated in the temperature t at M=11
per-pixel-scaled Chebyshev nodes t_m = Tk*u_m (Tk = max|k| per pixel):

    out_i = sum_m exp(u_m * Tk*v_i) * S_m,   S_m = R_m / G_m
    G_m   = sum_i exp(u_m * Tk*v_i)
    R_m   = sum_r Lc[r,m] * That_r,  That_r = sum_j T_r(k_j/Tk) q_j

M=11 includes the center node u=0 whose grid is identically 1: no exp pass,
no eval multiply; its S broadcast feeds the output sum directly.

Host sorts pixels by A = max|v|*max|k|; the hardest 128 per core go through
an exact pair-grid tile.

Performance structure (v8):
  - R produced directly by Lc-weighted reduction stationaries.
  - chebyshev chain parity-split over z = 2*khat^2-1 (half the serial depth).
  - warmup/filler matmuls run in the unused partition rows of the reduce
    psum tile (same banks, disjoint partitions) to hold the PE clock at
    2.4 GHz through analysis.
  - eval: S broadcast (PE) -> fp16 SBUF copy (ACT) -> 2x DVE multiply ->
    pairwise fp16 DVE add-tree.  No identity-accumulate matmuls at all:
    the eval PE work is just the broadcasts, deeply pipelined through a
    4-buffer PSUM pool (the accumulator banks are freed by the tree).
  - exact-tile work is interleaved into engine slack around the S phase.
  - no Ln activations (reciprocal_approx_fast): one exp table load.
"""

import sys

sys.path.insert(0, "/opt/trn_rl_repo")

from contextlib import ExitStack

import ml_dtypes
import numpy as np

import concourse.bacc as bacc
import concourse.bass as bass
import concourse.tile as tile
from concourse import mybir
from concourse.bass_utils import run_bass_kernel_spmd

B, C, H, W = 2, 64, 128, 128
N_CORES = 8
NPIX = B * H * W            # 32768
M = 11                      # interp nodes (odd: center node u=0 is free)
MC = M // 2
NEX_CORE = 128              # exact pixels per core
NEZ_CORE = NPIX // N_CORES - NEX_CORE   # 3968 interp pixels per core
FD = NEZ_CORE // 2          # 1984 pixels per g-half
HFD = FD // 2               # 992 pixels per column-half
R2 = 2 * M
N_WARM = 24                 # PE warmup matmuls
N_JUNK_ANA = 4              # filler matmuls per analysis m-step
# (half, m) slots whose S stays in PSUM (DVE 1x mul, no ACT copy)
PSUM_MUL_SLOTS = {(0, 2), (1, 2), (0, 8), (1, 8)}

FP32 = mybir.dt.float32
FP16 = mybir.dt.float16
BF16 = mybir.dt.bfloat16
EXP = mybir.ActivationFunctionType.Exp

U_NODES = np.cos(np.pi * np.arange(M) / (M - 1))


def _lc_matrix():
    u = U_NODES
    Tn = np.cos(np.arange(M)[:, None] * np.arccos(np.clip(u, -1, 1))[None, :])
    return np.linalg.inv(Tn.T)


def build_kernel():
    nc = bacc.Bacc(
        "TRN2",
        target_bir_lowering=False,
        debug=False,
        enable_asserts=False,
        num_devices=N_CORES,
    )
    vp = nc.dram_tensor("vp", [128, FD], FP16, kind="ExternalInput").ap()
    k2t = nc.dram_tensor("k2t", [128, FD], FP16, kind="ExternalInput").ap()
    qt = nc.dram_tensor("qt", [128, FD], FP16, kind="ExternalInput").ap()
    u1t = nc.dram_tensor("u1t", [128, FD], FP16, kind="ExternalInput").ap()
    statR = nc.dram_tensor("statR", [128, M, R2], FP16, kind="ExternalInput").ap()
    statL = nc.dram_tensor("statL", [128, M, R2], FP16, kind="ExternalInput").ap()
    statB = nc.dram_tensor("statB", [R2, M, 128], FP16, kind="ExternalInput").ap()
    vE = nc.dram_tensor("vE", [128, C], FP32, kind="ExternalInput").ap()
    kE = nc.dram_tensor("kE", [128, C], FP32, kind="ExternalInput").ap()
    qE = nc.dram_tensor("qE", [128, C], FP32, kind="ExternalInput").ap()

    outm = nc.dram_tensor("outm", [128, FD], FP16, kind="ExternalOutput").ap()
    oute = nc.dram_tensor("oute", [128, C], FP32, kind="ExternalOutput").ap()

    CH = [0, 512, HFD]

    with tile.TileContext(nc) as tc, ExitStack() as ctx:
        sb = ctx.enter_context(tc.tile_pool(name="sb", bufs=1))
        sbw = ctx.enter_context(tc.tile_pool(name="sbw", bufs=1))
        sbp = ctx.enter_context(tc.tile_pool(name="sbp", bufs=2))

        wu = sbw.tile([128, 512], FP16, tag="wu")
        nc.vector.memset(wu, 1.0)

        # dummy activation: pull the exp table load off the critical path
        dum = sbw.tile([1, 16], FP32, tag="dum")
        nc.scalar.activation(out=dum, in_=wu[0:1, 0:16], func=EXP)

        # ---- input DMAs ----
        v_t = sb.tile([128, FD], FP16)
        k2_t = sb.tile([128, FD], FP16)
        q_t = sb.tile([128, FD], FP16)
        u1_t = sb.tile([128, FD], FP16)
        sR = sb.tile([128, M, R2], FP16)
        sL = sb.tile([128, M, R2], FP16)
        sB_t = sb.tile([R2, M, 128], FP16)
        nc.sync.dma_start(out=v_t[:, :FD // 2], in_=vp[:, :FD // 2])
        nc.sync.dma_start(out=k2_t, in_=k2t)
        nc.sync.dma_start(out=u1_t, in_=u1t)
        nc.sync.dma_start(out=q_t, in_=qt)
        nc.sync.dma_start(out=v_t[:, FD // 2:], in_=vp[:, FD // 2:])
        nc.gpsimd.dma_start(out=sR, in_=statR)
        nc.gpsimd.dma_start(out=sL, in_=statL)
        nc.gpsimd.dma_start(out=sB_t, in_=statB)

        ones_t = sbw.tile([128, FD], FP16, tag="ones")
        nc.vector.memset(ones_t, 1.0)

        # exact-tile small inputs (early: cheap, off the critical path)
        vE_t = sb.tile([128, C], FP32)
        nc.sync.dma_start(out=vE_t, in_=vE)
        kE_t = sb.tile([128, C], FP32)
        nc.sync.dma_start(out=kE_t, in_=kE)
        qE_t = sb.tile([128, C], FP32)
        nc.sync.dma_start(out=qE_t, in_=qE)
        v2 = sb.tile([128, C, 2], FP16)
        nc.scalar.copy(v2, vE_t[:, :, None].broadcast_to([128, C, 2]))
        kE16 = sb.tile([128, C], FP16)
        nc.scalar.copy(kE16, kE_t)

        # ---- X grids (ACT, fp16), per column-half for early start ----
        Xs = {}
        for m in range(M):
            if m == MC:
                Xs[m] = ones_t
                continue
            xm = sbw.tile([128, FD], FP16, tag=f"x{m}")
            nc.scalar.activation(out=xm[:, :FD // 2], in_=v_t[:, :FD // 2],
                                 func=EXP, scale=float(U_NODES[m]))
            nc.scalar.activation(out=xm[:, FD // 2:], in_=v_t[:, FD // 2:],
                                 func=EXP, scale=float(U_NODES[m]))
            Xs[m] = xm

        # ---- chebyshev chain, parity split over zz = 2z = k2^2 - 2 ----
        zz = sbw.tile([128, FD], FP16, tag="zz")
        nc.vector.tensor_mul(zz, k2_t, k2_t)
        nc.vector.tensor_scalar_add(zz, zz, -2.0)
        Us = {0: q_t, 1: u1_t}
        e1 = sbw.tile([128, FD], FP16, tag="e1")
        nc.vector.scalar_tensor_tensor(
            out=e1, in0=zz, scalar=0.5, in1=q_t,
            op0=mybir.AluOpType.mult, op1=mybir.AluOpType.mult)
        Us[2] = e1
        o1 = sbw.tile([128, FD], FP16, tag="o1")
        nc.vector.scalar_tensor_tensor(
            out=o1, in0=zz, scalar=-1.0, in1=u1_t,
            op0=mybir.AluOpType.add, op1=mybir.AluOpType.mult)
        Us[3] = o1
        for par in (0, 1):
            prev, cur = Us[0 + par], Us[2 + par]
            for r in range(4 + par, M, 2):
                tmp = sbp.tile([128, FD], FP16, tag=f"tmp{par}")
                nc.vector.tensor_mul(tmp, zz, cur)
                nxt = sbw.tile([128, FD], FP16, tag=f"u{r}")
                nc.vector.tensor_sub(nxt, tmp, prev)
                Us[r] = nxt
                prev, cur = cur, nxt

        # exact pair grid (DVE work early; ACT exp deferred to S phase)
        P_t = sb.tile([128, C, C], FP16)
        k_op = bass.AP(
            tensor=kE16.tensor, offset=kE16.offset,
            ap=[kE16.ap[0], [0, C], [2, C // 2], [1, 2]],
        )
        v_op = bass.AP(
            tensor=v2.tensor, offset=v2.offset,
            ap=[v2.ap[0], [2, C], [0, C // 2], [1, 2]],
        )
        nc.vector.tensor_mul(
            P_t.rearrange("p i (jh jp) -> p i jh jp", jp=2), k_op, v_op)

        # ---- analysis reductions: R rows 0:22 (Lc-weighted), G rows 32:54
        # warmup + filler matmuls live in rows 64:128 of the same tiles.
        with tc.tile_pool(name="red", bufs=2, space="PSUM") as redp:
            red_ts = [redp.tile([128, HFD], FP32, tag="red", name=f"red{h}")
                      for h in range(2)]
            _junk_i = [0]

            def junk_mm(n):
                for _ in range(n):
                    t = red_ts[_junk_i[0] % 2]
                    _junk_i[0] += 1
                    nc.tensor.matmul(t[64:128, 0:496], wu[:, 0:64],
                                     wu[:, 0:496], start=True, stop=True,
                                     skip_group_check=True)

            junk_mm(N_WARM)
            for m in range(M):
                for h in range(2):
                    hs = slice(h * HFD, (h + 1) * HFD)
                    r_ps = red_ts[h][0:R2, :]
                    g_ps = red_ts[h][32:32 + R2, :]
                    for a, b in zip(CH[:-1], CH[1:]):
                        nc.tensor.matmul(r_ps[:, a:b], sL[:, m, :],
                                         Us[m][:, hs][:, a:b],
                                         start=(m == 0), stop=(m == M - 1))
                        nc.tensor.matmul(g_ps[:, a:b], sR[:, m, :],
                                         Xs[m][:, hs][:, a:b],
                                         start=(m == 0), stop=(m == M - 1))
                junk_mm(N_JUNK_ANA)

            # ---- S = R / G per half (highest scheduler priority: the
            # whole eval stream is gated on these few small ops) ----
            s_halves = []
            for h in range(2):
                r_ps = red_ts[h][0:R2, :]
                g_ps = red_ts[h][32:32 + R2, :]
                gsb = sbw.tile([R2, HFD], FP32, tag=f"gsb{h}")
                nc.scalar.copy(gsb, g_ps)
                junk_mm(7)
                ginv = sbw.tile([R2, HFD], FP32, tag=f"ginv{h}")
                nc.vector.reciprocal_approx_fast(out=ginv, in_=gsb)
                s_th = sbw.tile([R2, HFD], FP16, tag=f"s{h}")
                nc.vector.tensor_mul(s_th, ginv, r_ps)
                s_halves.append(s_th)
                junk_mm(4)

        # exact-tile exp: scalar-engine slack right after the X stream
        E_t = sb.tile([128, C, C], BF16)
        for eb in range(4):
            nc.scalar.activation(out=E_t[:, eb * 16:(eb + 1) * 16, :],
                                 in_=P_t[:, eb * 16:(eb + 1) * 16, :],
                                 func=EXP)

        # ---- eval: out = sum_m X_m * bcast(S_m), fp16 DVE add-tree ----
        with tc.tile_pool(name="evp", bufs=4, space="PSUM") as evp:
            pend = {0: [], 1: []}   # binary-counter tree: (level, tile)

            def tree_push(half, t, level=0):
                pend[half].append((level, t))
                while (len(pend[half]) >= 2
                       and pend[half][-1][0] == pend[half][-2][0]):
                    l1, a2 = pend[half].pop()
                    _, b2 = pend[half].pop()
                    nc.vector.tensor_add(a2, a2, b2)
                    pend[half].append((l1 + 1, a2))

            def eval_slot(m, half):
                s_h = s_halves[half]
                s_b = evp.tile([128, HFD], FP32, tag="sbps",
                               name=f"sb{half}_{m}")
                for a, b in zip(CH[:-1], CH[1:]):
                    nc.tensor.matmul(s_b[:, a:b], sB_t[:, m, :],
                                     s_h[:, a:b], start=True, stop=True)
                if m == MC:
                    s_bs = sbp.tile([128, HFD], FP16, tag="sbs", bufs=6)
                    nc.scalar.copy(s_bs, s_b)
                    tree_push(half, s_bs)
                    return
                prod = sbp.tile([128, HFD], FP16, tag="prod", bufs=8)
                if (half, m) in PSUM_MUL_SLOTS:
                    nc.vector.tensor_mul(
                        prod, Xs[m][:, half * HFD:(half + 1) * HFD], s_b)
                else:
                    s_bs = sbp.tile([128, HFD], FP16, tag="sbs", bufs=6)
                    nc.scalar.copy(s_bs, s_b)
                    nc.vector.tensor_mul(
                        prod, Xs[m][:, half * HFD:(half + 1) * HFD], s_bs)
                tree_push(half, prod)

            # order: MC first (cheap), then the rest; halves interleaved
            order = [MC] + [m for m in range(M) if m != MC]
            for i, m in enumerate(order):
                for half in range(2):
                    eval_slot(m, half)
                if i == 1:
                    G1 = sb.tile([128, C // 4, C], BF16)
                    G2 = sb.tile([128, C // 4, C], BF16)
                    nc.vector.tensor_add(G1, E_t[:, : C // 4, :],
                                         E_t[:, C // 4: C // 2, :])
                    nc.vector.tensor_add(G2, E_t[:, C // 2: 3 * C // 4, :],
                                         E_t[:, 3 * C // 4:, :])
                    nc.vector.tensor_add(G1, G1, G2)
                if i == 3:
                    nc.vector.tensor_add(G1[:, : C // 8, :], G1[:, : C // 8, :],
                                         G1[:, C // 8: C // 4, :])
                    nc.vector.tensor_add(G1[:, : C // 16, :],
                                         G1[:, : C // 16, :],
                                         G1[:, C // 16: C // 8, :])
                    d_t = sb.tile([128, C], FP32)
                    nc.vector.tensor_reduce(
                        out=d_t, in_=G1[:, : C // 16, :].transpose([0, 2, 1]),
                        axis=mybir.AxisListType.X, op=mybir.AluOpType.add,
                    )
                    r_t = sb.tile([128, C], FP32)
                    nc.vector.reciprocal_approx_fast(out=r_t, in_=d_t)
                    w16 = sb.tile([128, C], BF16)
                    nc.vector.tensor_mul(w16, qE_t, r_t)
                if i == 4:
                    Q4 = C // 4
                    F1 = sb.tile([128, C, Q4], BF16)
                    F2 = sb.tile([128, C, Q4], BF16)
                    F3 = sb.tile([128, C, Q4], BF16)
                    F4 = sb.tile([128, C, Q4], BF16)
                    for fi, Fq in enumerate((F1, F2, F3, F4)):
                        nc.vector.tensor_mul(
                            Fq, E_t[:, :, fi * Q4: (fi + 1) * Q4],
                            w16[:, None, fi * Q4: (fi + 1) * Q4]
                            .broadcast_to([128, C, Q4]),
                        )

                if i == 8:
                    nc.vector.tensor_add(F1, F1, F2)
                    nc.vector.tensor_add(F3, F3, F4)
                    nc.vector.tensor_add(F1, F1, F3)
                    nc.vector.tensor_add(F1[:, :, : Q4 // 2],
                                         F1[:, :, : Q4 // 2],
                                         F1[:, :, Q4 // 2:])
                    nc.vector.tensor_add(F1[:, :, : Q4 // 4],
                                         F1[:, :, : Q4 // 4],
                                         F1[:, :, Q4 // 4: Q4 // 2])
                    oE = sb.tile([128, C], FP32)
                    nc.vector.tensor_reduce(
                        out=oE, in_=F1[:, :, : Q4 // 4],
                        axis=mybir.AxisListType.X, op=mybir.AluOpType.add,
                    )
                    nc.sync.dma_start(out=oute, in_=oE)

            # drain each half's tree; final merge per column-chunk with
            # the output DMA of each chunk issued immediately
            for half in range(2):
                while len(pend[half]) > 2:
                    _, a2 = pend[half].pop()
                    l2, b2 = pend[half].pop()
                    nc.vector.tensor_add(a2, a2, b2)
                    pend[half].append((l2 + 1, a2))
                if len(pend[half]) == 2:
                    _, rootA = pend[half].pop()
                    _, rootB = pend[half].pop()
                    for qd in range(2):
                        ss = slice(qd * 496, (qd + 1) * 496)
                        gs = slice(half * HFD + qd * 496,
                                   half * HFD + (qd + 1) * 496)
                        nc.vector.tensor_add(rootA[:, ss], rootA[:, ss],
                                             rootB[:, ss])
                        nc.sync.dma_start(out=outm[:, gs], in_=rootA[:, ss])
                else:
                    nc.sync.dma_start(
                        out=outm[:, half * HFD:(half + 1) * HFD],
                        in_=pend[half][0][1])

    nc.compile()
    return nc


_NC_CACHE = None


def _get_nc():
    global _NC_CACHE
    if _NC_CACHE is None:
        _NC_CACHE = build_kernel()
    return _NC_CACHE


def _prep(x, y, z):
    """Host prep: sort by difficulty, shard, scale. Returns in_maps + meta."""
    q = np.ascontiguousarray(np.transpose(np.asarray(x), (0, 2, 3, 1))).reshape(-1, C)
    k = np.ascontiguousarray(np.transpose(np.asarray(y), (0, 2, 3, 1))).reshape(-1, C)
    v = np.ascontiguousarray(np.transpose(np.asarray(z), (0, 2, 3, 1))).reshape(-1, C)
    Tk = np.abs(k).max(axis=1)
    A = Tk * np.abs(v).max(axis=1)
    order = np.argsort(A, kind="stable")
    easy = order[: NEZ_CORE * N_CORES]
    hard = order[NEZ_CORE * N_CORES:]

    Lc = _lc_matrix()
    statR = np.zeros((128, M, R2), np.float32)
    for m in range(M):
        for g in range(2):
            statR[g * 64:(g + 1) * 64, m, 2 * m + g] = 1
    statL = np.zeros((128, M, R2), np.float32)
    for r in range(M):
        for m in range(M):
            for g in range(2):
                statL[g * 64:(g + 1) * 64, r, 2 * m + g] = Lc[r, m]
    statB = np.zeros((R2, M, 128), np.float32)
    for m in range(M):
        for g in range(2):
            statB[2 * m + g, m, g * 64:(g + 1) * 64] = 1

    in_maps = []
    meta = []
    for c in range(N_CORES):
        ez = easy[c::N_CORES]
        hd = hard[c::N_CORES]
        kh = k[ez] / Tk[ez, None]
        vp_c = (Tk[ez, None] * v[ez]).astype(np.float16)
        k2_c = (2.0 * kh).astype(np.float16)
        q_c = q[ez].astype(np.float16)
        u1_c = (kh * q[ez]).astype(np.float16)

        def cmaj(a2d, dt):
            h0 = a2d[:FD].T
            h1 = a2d[FD:].T
            return np.ascontiguousarray(np.concatenate([h0, h1], axis=0)).astype(dt)

        in_maps.append({
            "vp": cmaj(vp_c, np.float16),
            "k2t": cmaj(k2_c, np.float16),
            "qt": cmaj(q_c, np.float16),
            "u1t": cmaj(u1_c, np.float16),
            "statR": statR.astype(np.float16),
            "statL": statL.astype(np.float16),
            "statB": statB.astype(np.float16),
            "vE": v[hd].astype(np.float32),
            "kE": k[hd].astype(np.float32),
            "qE": q[hd].astype(np.float32),
        })
        meta.append((ez, hd))
    return in_maps, meta


def kernel(x, y, z):
    nc = _get_nc()
    in_maps, meta = _prep(x, y, z)
    res = run_bass_kernel_spmd(nc, in_maps, core_ids=list(range(N_CORES)))
    out = np.empty((NPIX, C), np.float32)
    for c in range(N_CORES):
        ez, hd = meta[c]
        om = res.results[c]["outm"].astype(np.float32)
        out[ez[:FD]] = om[:64].T
        out[ez[FD:]] = om[64:].T
        out[hd] = res.results[c]["oute"]
    return np.ascontiguousarray(
        np.transpose(out.reshape(B, H, W, C), (0, 3, 1, 2))
    ).astype(np.float32)


# revision 40
# speedup vs baseline: 1.2130x; 1.1840x over previous
"""Trainium2 kernel: per-pixel channel-mixing attention via temperature
interpolation (sigma-interp), v8.

Math per pixel: out_i = sum_j sigma_i(k_j) q_j where sigma(t) = softmax(t*v)
over channels. sigma(t*v) is interpolated in the temperature t at M=11
per-pixel-scaled Chebyshev nodes t_m = Tk*u_m (Tk = max|k| per pixel):

    out_i = sum_m exp(u_m * Tk*v_i) * S_m,   S_m = R_m / G_m
    G_m   = sum_i exp(u_m * Tk*v_i)
    R_m   = sum_r Lc[r,m] * That_r,  That_r = sum_j T_r(k_j/Tk) q_j

M=11 includes the center node u=0 whose grid is identically 1: no exp pass,
no eval multiply; its S broadcast feeds the output sum directly.

Host sorts pixels by A = max|v|*max|k|; the hardest 128 per core go through
an exact pair-grid tile.

Performance structure (v8):
  - R produced directly by Lc-weighted reduction stationaries.
  - chebyshev chain parity-split over z = 2*khat^2-1 (half the serial depth).
  - warmup/filler matmuls run in the unused partition rows of the reduce
    psum tile (same banks, disjoint partitions) to hold the PE clock at
    2.4 GHz through analysis.
  - eval: S broadcast (PE) -> fp16 SBUF copy (ACT) -> 2x DVE multiply ->
    pairwise fp16 DVE add-tree.  No identity-accumulate matmuls at all:
    the eval PE work is just the broadcasts, deeply pipelined through a
    4-buffer PSUM pool (the accumulator banks are freed by the tree).
  - exact-tile work is interleaved into engine slack around the S phase.
  - no Ln activations (reciprocal_approx_fast): one exp table load.
"""

import sys

sys.path.insert(0, "/opt/trn_rl_repo")

from contextlib import ExitStack

import ml_dtypes
import numpy as np

import concourse.bacc as bacc
import concourse.bass as bass
import concourse.tile as tile
from concourse import mybir
from concourse.bass_utils import run_bass_kernel_spmd

B, C, H, W = 2, 64, 128, 128
N_CORES = 8
NPIX = B * H * W            # 32768
M = 11                      # interp nodes (odd: center node u=0 is free)
MC = M // 2
NEX_CORE = 128              # exact pixels per core
NEZ_CORE = NPIX // N_CORES - NEX_CORE   # 3968 interp pixels per core
FD = NEZ_CORE // 2          # 1984 pixels per g-half
HFD = FD // 2               # 992 pixels per column-half
R2 = 2 * M
N_WARM = 24                 # PE warmup matmuls
N_JUNK_ANA = 4              # filler matmuls per analysis m-step
# (half, m) slots whose S stays in PSUM (DVE 1x mul, no ACT copy)
PSUM_MUL_SLOTS = {(0, 2), (1, 2), (0, 8), (1, 8)}

FP32 = mybir.dt.float32
FP16 = mybir.dt.float16
BF16 = mybir.dt.bfloat16
EXP = mybir.ActivationFunctionType.Exp

U_NODES = np.cos(np.pi * np.arange(M) / (M - 1))


def _lc_matrix():
    u = U_NODES
    Tn = np.cos(np.arange(M)[:, None] * np.arccos(np.clip(u, -1, 1))[None, :])
    return np.linalg.inv(Tn.T)


def build_kernel():
    nc = bacc.Bacc(
        "TRN2",
        target_bir_lowering=False,
        debug=False,
        enable_asserts=False,
        num_devices=N_CORES,
    )
    vp = nc.dram_tensor("vp", [128, FD], FP16, kind="ExternalInput").ap()
    k2t = nc.dram_tensor("k2t", [128, FD], FP16, kind="ExternalInput").ap()
    qt = nc.dram_tensor("qt", [128, FD], FP16, kind="ExternalInput").ap()
    u1t = nc.dram_tensor("u1t", [128, FD], FP16, kind="ExternalInput").ap()
    statR = nc.dram_tensor("statR", [128, M, R2], FP16, kind="ExternalInput").ap()
    statL = nc.dram_tensor("statL", [128, M, R2], FP16, kind="ExternalInput").ap()
    statB = nc.dram_tensor("statB", [R2, M, 128], FP16, kind="ExternalInput").ap()
    vE = nc.dram_tensor("vE", [128, C], FP32, kind="ExternalInput").ap()
    kE = nc.dram_tensor("kE", [128, C], FP32, kind="ExternalInput").ap()
    qE = nc.dram_tensor("qE", [128, C], FP32, kind="ExternalInput").ap()

    outm = nc.dram_tensor("outm", [128, FD], FP16, kind="ExternalOutput").ap()
    oute = nc.dram_tensor("oute", [128, C], FP32, kind="ExternalOutput").ap()

    CH = [0, 512, HFD]

    with tile.TileContext(nc) as tc, ExitStack() as ctx:
        sb = ctx.enter_context(tc.tile_pool(name="sb", bufs=1))
        sbw = ctx.enter_context(tc.tile_pool(name="sbw", bufs=1))
        sbp = ctx.enter_context(tc.tile_pool(name="sbp", bufs=2))

        wu = sbw.tile([128, 512], FP16, tag="wu")
        nc.vector.memset(wu, 1.0)

        # dummy activation: pull the exp table load off the critical path
        dum = sbw.tile([1, 16], FP32, tag="dum")
        nc.scalar.activation(out=dum, in_=wu[0:1, 0:16], func=EXP)

        # ---- input DMAs ----
        v_t = sb.tile([128, FD], FP16)
        k2_t = sb.tile([128, FD], FP16)
        q_t = sb.tile([128, FD], FP16)
        u1_t = sb.tile([128, FD], FP16)
        sR = sb.tile([128, M, R2], FP16)
        sL = sb.tile([128, M, R2], FP16)
        sB_t = sb.tile([R2, M, 128], FP16)
        nc.sync.dma_start(out=v_t[:, :FD // 2], in_=vp[:, :FD // 2])
        nc.sync.dma_start(out=k2_t, in_=k2t)
        nc.sync.dma_start(out=u1_t, in_=u1t)
        nc.sync.dma_start(out=q_t, in_=qt)
        nc.sync.dma_start(out=v_t[:, FD // 2:], in_=vp[:, FD // 2:])
        nc.gpsimd.dma_start(out=sR, in_=statR)
        nc.gpsimd.dma_start(out=sL, in_=statL)
        nc.gpsimd.dma_start(out=sB_t, in_=statB)

        ones_t = sbw.tile([128, FD], FP16, tag="ones")
        nc.vector.memset(ones_t, 1.0)

        # exact-tile small inputs (early: cheap, off the critical path)
        vE_t = sb.tile([128, C], FP32)
        nc.sync.dma_start(out=vE_t, in_=vE)
        kE_t = sb.tile([128, C], FP32)
        nc.sync.dma_start(out=kE_t, in_=kE)
        qE_t = sb.tile([128, C], FP32)
        nc.sync.dma_start(out=qE_t, in_=qE)
        v2 = sb.tile([128, C, 2], FP16)
        nc.scalar.copy(v2, vE_t[:, :, None].broadcast_to([128, C, 2]))
        kE16 = sb.tile([128, C], FP16)
        nc.scalar.copy(kE16, kE_t)

        # ---- X grids (ACT, fp16), per column-half for early start ----
        Xs = {}
        for m in range(M):
            if m == MC:
                Xs[m] = ones_t
                continue
            xm = sbw.tile([128, FD], FP16, tag=f"x{m}")
            nc.scalar.activation(out=xm[:, :FD // 2], in_=v_t[:, :FD // 2],
                                 func=EXP, scale=float(U_NODES[m]))
            nc.scalar.activation(out=xm[:, FD // 2:], in_=v_t[:, FD // 2:],
                                 func=EXP, scale=float(U_NODES[m]))
            Xs[m] = xm

        # ---- chebyshev chain, parity split over zz = 2z = k2^2 - 2 ----
        zz = sbw.tile([128, FD], FP16, tag="zz")
        nc.vector.tensor_mul(zz, k2_t, k2_t)
        nc.vector.tensor_scalar_add(zz, zz, -2.0)
        Us = {0: q_t, 1: u1_t}
        e1 = sbw.tile([128, FD], FP16, tag="e1")
        nc.vector.scalar_tensor_tensor(
            out=e1, in0=zz, scalar=0.5, in1=q_t,
            op0=mybir.AluOpType.mult, op1=mybir.AluOpType.mult)
        Us[2] = e1
        o1 = sbw.tile([128, FD], FP16, tag="o1")
        nc.vector.scalar_tensor_tensor(
            out=o1, in0=zz, scalar=-1.0, in1=u1_t,
            op0=mybir.AluOpType.add, op1=mybir.AluOpType.mult)
        Us[3] = o1
        for par in (0, 1):
            prev, cur = Us[0 + par], Us[2 + par]
            for r in range(4 + par, M, 2):
                tmp = sbp.tile([128, FD], FP16, tag=f"tmp{par}")
                nc.vector.tensor_mul(tmp, zz, cur)
                nxt = sbw.tile([128, FD], FP16, tag=f"u{r}")
                nc.vector.tensor_sub(nxt, tmp, prev)
                Us[r] = nxt
                prev, cur = cur, nxt

        # exact pair grid (DVE work early; ACT exp deferred to S phase)
        P_t = sb.tile([128, C, C], FP16)
        k_op = bass.AP(
            tensor=kE16.tensor, offset=kE16.offset,
            ap=[kE16.ap[0], [0, C], [2, C // 2], [1, 2]],
        )
        v_op = bass.AP(
            tensor=v2.tensor, offset=v2.offset,
            ap=[v2.ap[0], [2, C], [0, C // 2], [1, 2]],
        )
        nc.vector.tensor_mul(
            P_t.rearrange("p i (jh jp) -> p i jh jp", jp=2), k_op, v_op)

        # ---- analysis reductions: R rows 0:22 (Lc-weighted), G rows 32:54
        # warmup + filler matmuls live in rows 64:128 of the same tiles.
        with tc.tile_pool(name="red", bufs=2, space="PSUM") as redp:
            red_ts = [redp.tile([128, HFD], FP32, tag="red", name=f"red{h}")
                      for h in range(2)]
            _junk_i = [0]

            def junk_mm(n):
                for _ in range(n):
                    t = red_ts[_junk_i[0] % 2]
                    _junk_i[0] += 1
                    nc.tensor.matmul(t[64:128, 0:496], wu[:, 0:64],
                                     wu[:, 0:496], start=True, stop=True,
                                     skip_group_check=True)

            junk_mm(N_WARM)
            for m in range(M):
                for h in range(2):
                    hs = slice(h * HFD, (h + 1) * HFD)
                    r_ps = red_ts[h][0:R2, :]
                    g_ps = red_ts[h][32:32 + R2, :]
                    for a, b in zip(CH[:-1], CH[1:]):
                        nc.tensor.matmul(r_ps[:, a:b], sL[:, m, :],
                                         Us[m][:, hs][:, a:b],
                                         start=(m == 0), stop=(m == M - 1))
                        nc.tensor.matmul(g_ps[:, a:b], sR[:, m, :],
                                         Xs[m][:, hs][:, a:b],
                                         start=(m == 0), stop=(m == M - 1))
                junk_mm(N_JUNK_ANA)

            # ---- S = R / G per half (highest scheduler priority: the
            # whole eval stream is gated on these few small ops) ----
            s_halves = []
            for h in range(2):
                r_ps = red_ts[h][0:R2, :]
                g_ps = red_ts[h][32:32 + R2, :]
                gsb = sbw.tile([R2, HFD], FP32, tag=f"gsb{h}")
                nc.scalar.copy(gsb, g_ps)
                junk_mm(7)
                ginv = sbw.tile([R2, HFD], FP32, tag=f"ginv{h}")
                nc.vector.reciprocal_approx_fast(out=ginv, in_=gsb)
                s_th = sbw.tile([R2, HFD], FP16, tag=f"s{h}")
                nc.vector.tensor_mul(s_th, ginv, r_ps)
                s_halves.append(s_th)
                junk_mm(4)

        # exact-tile exp: scalar-engine slack right after the X stream
        E_t = sb.tile([128, C, C], BF16)
        for eb in range(4):
            nc.scalar.activation(out=E_t[:, eb * 16:(eb + 1) * 16, :],
                                 in_=P_t[:, eb * 16:(eb + 1) * 16, :],
                                 func=EXP)

        # ---- eval: out = sum_m X_m * bcast(S_m), fp16 DVE add-tree ----
        with tc.tile_pool(name="evp", bufs=4, space="PSUM") as evp:
            pend = {0: [], 1: []}   # binary-counter tree: (level, tile)

            def tree_push(half, t, level=0):
                pend[half].append((level, t))
                while (len(pend[half]) >= 2
                       and pend[half][-1][0] == pend[half][-2][0]):
                    l1, a2 = pend[half].pop()
                    _, b2 = pend[half].pop()
                    nc.vector.tensor_add(a2, a2, b2)
                    pend[half].append((l1 + 1, a2))

            def eval_slot(m, half):
                s_h = s_halves[half]
                s_b = evp.tile([128, HFD], FP32, tag="sbps",
                               name=f"sb{half}_{m}")
                for a, b in zip(CH[:-1], CH[1:]):
                    nc.tensor.matmul(s_b[:, a:b], sB_t[:, m, :],
                                     s_h[:, a:b], start=True, stop=True)
                if m == MC:
                    s_bs = sbp.tile([128, HFD], FP16, tag="sbs", bufs=6)
                    nc.scalar.copy(s_bs, s_b)
                    tree_push(half, s_bs)
                    return
                prod = sbp.tile([128, HFD], FP16, tag="prod", bufs=8)
                if (half, m) in PSUM_MUL_SLOTS:
                    nc.vector.tensor_mul(
                        prod, Xs[m][:, half * HFD:(half + 1) * HFD], s_b)
                else:
                    s_bs = sbp.tile([128, HFD], FP16, tag="sbs", bufs=6)
                    nc.scalar.copy(s_bs, s_b)
                    nc.vector.tensor_mul(
                        prod, Xs[m][:, half * HFD:(half + 1) * HFD], s_bs)
                tree_push(half, prod)

            # order: MC first (cheap), then the rest; halves interleaved
            order = [MC] + [m for m in range(M) if m != MC]
            for i, m in enumerate(order):
                for half in range(2):
                    eval_slot(m, half)
                if i == 1:
                    G1 = sb.tile([128, C // 4, C], BF16)
                    G2 = sb.tile([128, C // 4, C], BF16)
                    nc.vector.tensor_add(G1, E_t[:, : C // 4, :],
                                         E_t[:, C // 4: C // 2, :])
                    nc.vector.tensor_add(G2, E_t[:, C // 2: 3 * C // 4, :],
                                         E_t[:, 3 * C // 4:, :])
                    nc.vector.tensor_add(G1, G1, G2)
                if i == 3:
                    nc.vector.tensor_add(G1[:, : C // 8, :], G1[:, : C // 8, :],
                                         G1[:, C // 8: C // 4, :])
                    nc.vector.tensor_add(G1[:, : C // 16, :],
                                         G1[:, : C // 16, :],
                                         G1[:, C // 16: C // 8, :])
                    d_t = sb.tile([128, C], FP32)
                    nc.vector.tensor_reduce(
                        out=d_t, in_=G1[:, : C // 16, :].transpose([0, 2, 1]),
                        axis=mybir.AxisListType.X, op=mybir.AluOpType.add,
                    )
                    r_t = sb.tile([128, C], FP32)
                    nc.vector.reciprocal_approx_fast(out=r_t, in_=d_t)
                    w16 = sb.tile([128, C], BF16)
                    nc.vector.tensor_mul(w16, qE_t, r_t)
                if i == 5:
                    Q4 = C // 4
                    F1 = sb.tile([128, C, Q4], BF16)
                    F2 = sb.tile([128, C, Q4], BF16)
                    F3 = sb.tile([128, C, Q4], BF16)
                    F4 = sb.tile([128, C, Q4], BF16)
                    for fi, Fq in enumerate((F1, F2, F3, F4)):
                        nc.vector.tensor_mul(
                            Fq, E_t[:, :, fi * Q4: (fi + 1) * Q4],
                            w16[:, None, fi * Q4: (fi + 1) * Q4]
                            .broadcast_to([128, C, Q4]),
                        )
                    nc.vector.tensor_add(F1, F1, F2)
                    nc.vector.tensor_add(F3, F3, F4)

            # drain each half's tree; final merge per column-chunk with
            # the output DMA of each chunk issued immediately
            for half in range(2):
                while len(pend[half]) > 2:
                    _, a2 = pend[half].pop()
                    l2, b2 = pend[half].pop()
                    nc.vector.tensor_add(a2, a2, b2)
                    pend[half].append((l2 + 1, a2))
                if len(pend[half]) == 2:
                    _, rootA = pend[half].pop()
                    _, rootB = pend[half].pop()
                    for qd in range(2):
                        ss = slice(qd * 496, (qd + 1) * 496)
                        gs = slice(half * HFD + qd * 496,
                                   half * HFD + (qd + 1) * 496)
                        nc.vector.tensor_add(rootA[:, ss], rootA[:, ss],
                                             rootB[:, ss])
                        nc.sync.dma_start(out=outm[:, gs], in_=rootA[:, ss])
                else:
                    nc.sync.dma_start(
                        out=outm[:, half * HFD:(half + 1) * HFD],
                        in_=pend[half][0][1])


            # exact-tile numerator tail (overlaps the outm DMAs)
            nc.vector.tensor_add(F1, F1, F3)
            Q4 = C // 4
            nc.vector.tensor_add(F1[:, :, : Q4 // 2], F1[:, :, : Q4 // 2],
                                 F1[:, :, Q4 // 2:])
            nc.vector.tensor_add(F1[:, :, : Q4 // 4], F1[:, :, : Q4 // 4],
                                 F1[:, :, Q4 // 4: Q4 // 2])
            oE = sb.tile([128, C], FP32)
            nc.vector.tensor_reduce(
                out=oE, in_=F1[:, :, : Q4 // 4],
                axis=mybir.AxisListType.X, op=mybir.AluOpType.add,
            )
            nc.sync.dma_start(out=oute, in_=oE)

    nc.compile()
    return nc


_NC_CACHE = None


def _get_nc():
    global _NC_CACHE
    if _NC_CACHE is None:
        _NC_CACHE = build_kernel()
    return _NC_CACHE


def _prep(x, y, z):
    """Host prep: sort by difficulty, shard, scale. Returns in_maps + meta."""
    q = np.ascontiguousarray(np.transpose(np.asarray(x), (0, 2, 3, 1))).reshape(-1, C)
    k = np.ascontiguousarray(np.transpose(np.asarray(y), (0, 2, 3, 1))).reshape(-1, C)
    v = np.ascontiguousarray(np.transpose(np.asarray(z), (0, 2, 3, 1))).reshape(-1, C)
    Tk = np.abs(k).max(axis=1)
    A = Tk * np.abs(v).max(axis=1)
    order = np.argsort(A, kind="stable")
    easy = order[: NEZ_CORE * N_CORES]
    hard = order[NEZ_CORE * N_CORES:]

    Lc = _lc_matrix()
    statR = np.zeros((128, M, R2), np.float32)
    for m in range(M):
        for g in range(2):
            statR[g * 64:(g + 1) * 64, m, 2 * m + g] = 1
    statL = np.zeros((128, M, R2), np.float32)
    for r in range(M):
        for m in range(M):
            for g in range(2):
                statL[g * 64:(g + 1) * 64, r, 2 * m + g] = Lc[r, m]
    statB = np.zeros((R2, M, 128), np.float32)
    for m in range(M):
        for g in range(2):
            statB[2 * m + g, m, g * 64:(g + 1) * 64] = 1

    in_maps = []
    meta = []
    for c in range(N_CORES):
        ez = easy[c::N_CORES]
        hd = hard[c::N_CORES]
        kh = k[ez] / Tk[ez, None]
        vp_c = (Tk[ez, None] * v[ez]).astype(np.float16)
        k2_c = (2.0 * kh).astype(np.float16)
        q_c = q[ez].astype(np.float16)
        u1_c = (kh * q[ez]).astype(np.float16)

        def cmaj(a2d, dt):
            h0 = a2d[:FD].T
            h1 = a2d[FD:].T
            return np.ascontiguousarray(np.concatenate([h0, h1], axis=0)).astype(dt)

        in_maps.append({
            "vp": cmaj(vp_c, np.float16),
            "k2t": cmaj(k2_c, np.float16),
            "qt": cmaj(q_c, np.float16),
            "u1t": cmaj(u1_c, np.float16),
            "statR": statR.astype(np.float16),
            "statL": statL.astype(np.float16),
            "statB": statB.astype(np.float16),
            "vE": v[hd].astype(np.float32),
            "kE": k[hd].astype(np.float32),
            "qE": q[hd].astype(np.float32),
        })
        meta.append((ez, hd))
    return in_maps, meta


def kernel(x, y, z):
    nc = _get_nc()
    in_maps, meta = _prep(x, y, z)
    res = run_bass_kernel_spmd(nc, in_maps, core_ids=list(range(N_CORES)))
    out = np.empty((NPIX, C), np.float32)
    for c in range(N_CORES):
        ez, hd = meta[c]
        om = res.results[c]["outm"].astype(np.float32)
        out[ez[:FD]] = om[:64].T
        out[ez[FD:]] = om[64:].T
        out[hd] = res.results[c]["oute"]
    return np.ascontiguousarray(
        np.transpose(out.reshape(B, H, W, C), (0, 3, 1, 2))
    ).astype(np.float32)


# revision 41
# speedup vs baseline: 1.2256x; 1.0104x over previous
"""Trainium2 kernel: per-pixel channel-mixing attention via temperature
interpolation (sigma-interp), v8.

Math per pixel: out_i = sum_j sigma_i(k_j) q_j where sigma(t) = softmax(t*v)
over channels. sigma(t*v) is interpolated in the temperature t at M=11
per-pixel-scaled Chebyshev nodes t_m = Tk*u_m (Tk = max|k| per pixel):

    out_i = sum_m exp(u_m * Tk*v_i) * S_m,   S_m = R_m / G_m
    G_m   = sum_i exp(u_m * Tk*v_i)
    R_m   = sum_r Lc[r,m] * That_r,  That_r = sum_j T_r(k_j/Tk) q_j

M=11 includes the center node u=0 whose grid is identically 1: no exp pass,
no eval multiply; its S broadcast feeds the output sum directly.

Host sorts pixels by A = max|v|*max|k|; the hardest 128 per core go through
an exact pair-grid tile.

Performance structure (v8):
  - R produced directly by Lc-weighted reduction stationaries.
  - chebyshev chain parity-split over z = 2*khat^2-1 (half the serial depth).
  - warmup/filler matmuls run in the unused partition rows of the reduce
    psum tile (same banks, disjoint partitions) to hold the PE clock at
    2.4 GHz through analysis.
  - eval: S broadcast (PE) -> fp16 SBUF copy (ACT) -> 2x DVE multiply ->
    pairwise fp16 DVE add-tree.  No identity-accumulate matmuls at all:
    the eval PE work is just the broadcasts, deeply pipelined through a
    4-buffer PSUM pool (the accumulator banks are freed by the tree).
  - exact-tile work is interleaved into engine slack around the S phase.
  - no Ln activations (reciprocal_approx_fast): one exp table load.
"""

import sys

sys.path.insert(0, "/opt/trn_rl_repo")

from contextlib import ExitStack

import ml_dtypes
import numpy as np

import concourse.bacc as bacc
import concourse.bass as bass
import concourse.tile as tile
from concourse import mybir
from concourse.bass_utils import run_bass_kernel_spmd

B, C, H, W = 2, 64, 128, 128
N_CORES = 8
NPIX = B * H * W            # 32768
M = 11                      # interp nodes (odd: center node u=0 is free)
MC = M // 2
NEX_CORE = 128              # exact pixels per core
NEZ_CORE = NPIX // N_CORES - NEX_CORE   # 3968 interp pixels per core
FD = NEZ_CORE // 2          # 1984 pixels per g-half
HFD = FD // 2               # 992 pixels per column-half
R2 = 2 * M
N_WARM = 24                 # PE warmup matmuls
N_JUNK_ANA = 4              # filler matmuls per analysis m-step
# (half, m) slots whose S stays in PSUM (DVE 1x mul, no ACT copy)
PSUM_MUL_SLOTS = {(0, 2), (1, 2), (0, 8), (1, 8)}

FP32 = mybir.dt.float32
FP16 = mybir.dt.float16
BF16 = mybir.dt.bfloat16
EXP = mybir.ActivationFunctionType.Exp

U_NODES = np.cos(np.pi * np.arange(M) / (M - 1))


def _lc_matrix():
    u = U_NODES
    Tn = np.cos(np.arange(M)[:, None] * np.arccos(np.clip(u, -1, 1))[None, :])
    return np.linalg.inv(Tn.T)


def build_kernel():
    nc = bacc.Bacc(
        "TRN2",
        target_bir_lowering=False,
        debug=False,
        enable_asserts=False,
        num_devices=N_CORES,
    )
    vp = nc.dram_tensor("vp", [128, FD], FP16, kind="ExternalInput").ap()
    k2t = nc.dram_tensor("k2t", [128, FD], FP16, kind="ExternalInput").ap()
    qt = nc.dram_tensor("qt", [128, FD], FP16, kind="ExternalInput").ap()
    u1t = nc.dram_tensor("u1t", [128, FD], FP16, kind="ExternalInput").ap()
    statR = nc.dram_tensor("statR", [128, M, R2], FP16, kind="ExternalInput").ap()
    statL = nc.dram_tensor("statL", [128, M, R2], FP16, kind="ExternalInput").ap()
    statB = nc.dram_tensor("statB", [R2, M, 128], FP16, kind="ExternalInput").ap()
    vE = nc.dram_tensor("vE", [128, C], FP32, kind="ExternalInput").ap()
    kE = nc.dram_tensor("kE", [128, C], FP32, kind="ExternalInput").ap()
    qE = nc.dram_tensor("qE", [128, C], FP32, kind="ExternalInput").ap()

    outm = nc.dram_tensor("outm", [128, FD], FP16, kind="ExternalOutput").ap()
    oute = nc.dram_tensor("oute", [128, C], FP32, kind="ExternalOutput").ap()

    CH = [0, 512, HFD]

    with tile.TileContext(nc) as tc, ExitStack() as ctx:
        sb = ctx.enter_context(tc.tile_pool(name="sb", bufs=1))
        sbw = ctx.enter_context(tc.tile_pool(name="sbw", bufs=1))
        sbp = ctx.enter_context(tc.tile_pool(name="sbp", bufs=2))

        wu = sbw.tile([128, 512], FP16, tag="wu")
        nc.vector.memset(wu, 1.0)

        # dummy activation: pull the exp table load off the critical path
        dum = sbw.tile([1, 16], FP32, tag="dum")
        nc.scalar.activation(out=dum, in_=wu[0:1, 0:16], func=EXP)

        # ---- input DMAs ----
        v_t = sb.tile([128, FD], FP16)
        k2_t = sb.tile([128, FD], FP16)
        q_t = sb.tile([128, FD], FP16)
        u1_t = sb.tile([128, FD], FP16)
        sR = sb.tile([128, M, R2], FP16)
        sL = sb.tile([128, M, R2], FP16)
        sB_t = sb.tile([R2, M, 128], FP16)
        nc.sync.dma_start(out=v_t[:, :FD // 2], in_=vp[:, :FD // 2])
        nc.sync.dma_start(out=k2_t, in_=k2t)
        nc.sync.dma_start(out=u1_t, in_=u1t)
        nc.sync.dma_start(out=q_t, in_=qt)
        nc.sync.dma_start(out=v_t[:, FD // 2:], in_=vp[:, FD // 2:])
        nc.gpsimd.dma_start(out=sR, in_=statR)
        nc.gpsimd.dma_start(out=sL, in_=statL)
        nc.gpsimd.dma_start(out=sB_t, in_=statB)

        ones_t = sbw.tile([128, FD], FP16, tag="ones")
        nc.vector.memset(ones_t, 1.0)

        # exact-tile small inputs (early: cheap, off the critical path)
        vE_t = sb.tile([128, C], FP32)
        nc.sync.dma_start(out=vE_t, in_=vE)
        kE_t = sb.tile([128, C], FP32)
        nc.sync.dma_start(out=kE_t, in_=kE)
        qE_t = sb.tile([128, C], FP32)
        nc.sync.dma_start(out=qE_t, in_=qE)
        v2 = sb.tile([128, C, 2], FP16)
        nc.scalar.copy(v2, vE_t[:, :, None].broadcast_to([128, C, 2]))
        kE16 = sb.tile([128, C], FP16)
        nc.scalar.copy(kE16, kE_t)

        # ---- X grids (ACT, fp16), per column-half for early start ----
        Xs = {}
        for m in range(M):
            if m == MC:
                Xs[m] = ones_t
                continue
            xm = sbw.tile([128, FD], FP16, tag=f"x{m}")
            nc.scalar.activation(out=xm[:, :FD // 2], in_=v_t[:, :FD // 2],
                                 func=EXP, scale=float(U_NODES[m]))
            nc.scalar.activation(out=xm[:, FD // 2:], in_=v_t[:, FD // 2:],
                                 func=EXP, scale=float(U_NODES[m]))
            Xs[m] = xm

        # ---- chebyshev chain, parity split over zz = 2z = k2^2 - 2 ----
        zz = sbw.tile([128, FD], FP16, tag="zz")
        nc.vector.tensor_mul(zz, k2_t, k2_t)
        nc.vector.tensor_scalar_add(zz, zz, -2.0)
        Us = {0: q_t, 1: u1_t}
        e1 = sbw.tile([128, FD], FP16, tag="e1")
        nc.vector.scalar_tensor_tensor(
            out=e1, in0=zz, scalar=0.5, in1=q_t,
            op0=mybir.AluOpType.mult, op1=mybir.AluOpType.mult)
        Us[2] = e1
        o1 = sbw.tile([128, FD], FP16, tag="o1")
        nc.vector.scalar_tensor_tensor(
            out=o1, in0=zz, scalar=-1.0, in1=u1_t,
            op0=mybir.AluOpType.add, op1=mybir.AluOpType.mult)
        Us[3] = o1
        for par in (0, 1):
            prev, cur = Us[0 + par], Us[2 + par]
            for r in range(4 + par, M, 2):
                tmp = sbp.tile([128, FD], FP16, tag=f"tmp{par}")
                nc.vector.tensor_mul(tmp, zz, cur)
                nxt = sbw.tile([128, FD], FP16, tag=f"u{r}")
                nc.vector.tensor_sub(nxt, tmp, prev)
                Us[r] = nxt
                prev, cur = cur, nxt

        # exact pair grid (DVE work early; ACT exp deferred to S phase)
        P_t = sb.tile([128, C, C], FP16)
        k_op = bass.AP(
            tensor=kE16.tensor, offset=kE16.offset,
            ap=[kE16.ap[0], [0, C], [2, C // 2], [1, 2]],
        )
        v_op = bass.AP(
            tensor=v2.tensor, offset=v2.offset,
            ap=[v2.ap[0], [2, C], [0, C // 2], [1, 2]],
        )
        nc.vector.tensor_mul(
            P_t.rearrange("p i (jh jp) -> p i jh jp", jp=2), k_op, v_op)

        # ---- analysis reductions: R rows 0:22 (Lc-weighted), G rows 32:54
        # warmup + filler matmuls live in rows 64:128 of the same tiles.
        with tc.tile_pool(name="red", bufs=2, space="PSUM") as redp:
            red_ts = [redp.tile([128, HFD], FP32, tag="red", name=f"red{h}")
                      for h in range(2)]
            _junk_i = [0]

            def junk_mm(n):
                for _ in range(n):
                    t = red_ts[_junk_i[0] % 2]
                    _junk_i[0] += 1
                    nc.tensor.matmul(t[64:128, 0:496], wu[:, 0:64],
                                     wu[:, 0:496], start=True, stop=True,
                                     skip_group_check=True)

            junk_mm(N_WARM)
            for m in range(M):
                for h in range(2):
                    hs = slice(h * HFD, (h + 1) * HFD)
                    r_ps = red_ts[h][0:R2, :]
                    g_ps = red_ts[h][32:32 + R2, :]
                    for a, b in zip(CH[:-1], CH[1:]):
                        nc.tensor.matmul(r_ps[:, a:b], sL[:, m, :],
                                         Us[m][:, hs][:, a:b],
                                         start=(m == 0), stop=(m == M - 1))
                        nc.tensor.matmul(g_ps[:, a:b], sR[:, m, :],
                                         Xs[m][:, hs][:, a:b],
                                         start=(m == 0), stop=(m == M - 1))
                junk_mm(N_JUNK_ANA)

            # ---- S = R / G per half (highest scheduler priority: the
            # whole eval stream is gated on these few small ops) ----
            s_halves = []
            for h in range(2):
                r_ps = red_ts[h][0:R2, :]
                g_ps = red_ts[h][32:32 + R2, :]
                gsb = sbw.tile([R2, HFD], FP32, tag=f"gsb{h}")
                nc.scalar.copy(gsb, g_ps)
                junk_mm(2)
                ginv = sbw.tile([R2, HFD], FP32, tag=f"ginv{h}")
                nc.vector.reciprocal_approx_fast(out=ginv, in_=gsb)
                s_th = sbw.tile([R2, HFD], FP16, tag=f"s{h}")
                nc.vector.tensor_mul(s_th, ginv, r_ps)
                s_halves.append(s_th)
                junk_mm(1)

        # exact-tile exp: scalar-engine slack right after the X stream
        E_t = sb.tile([128, C, C], BF16)
        for eb in range(4):
            nc.scalar.activation(out=E_t[:, eb * 16:(eb + 1) * 16, :],
                                 in_=P_t[:, eb * 16:(eb + 1) * 16, :],
                                 func=EXP)

        # ---- eval: out = sum_m X_m * bcast(S_m), fp16 DVE add-tree ----
        with tc.tile_pool(name="evp", bufs=4, space="PSUM") as evp:
            pend = {0: [], 1: []}   # binary-counter tree: (level, tile)

            def tree_push(half, t, level=0):
                pend[half].append((level, t))
                while (len(pend[half]) >= 2
                       and pend[half][-1][0] == pend[half][-2][0]):
                    l1, a2 = pend[half].pop()
                    _, b2 = pend[half].pop()
                    nc.vector.tensor_add(a2, a2, b2)
                    pend[half].append((l1 + 1, a2))

            def eval_slot(m, half):
                s_h = s_halves[half]
                s_b = evp.tile([128, HFD], FP32, tag="sbps",
                               name=f"sb{half}_{m}")
                for a, b in zip(CH[:-1], CH[1:]):
                    nc.tensor.matmul(s_b[:, a:b], sB_t[:, m, :],
                                     s_h[:, a:b], start=True, stop=True)
                if m == MC:
                    s_bs = sbp.tile([128, HFD], FP16, tag="sbs", bufs=6)
                    nc.scalar.copy(s_bs, s_b)
                    tree_push(half, s_bs)
                    return
                prod = sbp.tile([128, HFD], FP16, tag="prod", bufs=8)
                if (half, m) in PSUM_MUL_SLOTS:
                    nc.vector.tensor_mul(
                        prod, Xs[m][:, half * HFD:(half + 1) * HFD], s_b)
                else:
                    s_bs = sbp.tile([128, HFD], FP16, tag="sbs", bufs=6)
                    nc.scalar.copy(s_bs, s_b)
                    nc.vector.tensor_mul(
                        prod, Xs[m][:, half * HFD:(half + 1) * HFD], s_bs)
                tree_push(half, prod)

            # order: MC first (cheap), then the rest; halves interleaved
            order = [MC] + [m for m in range(M) if m != MC]
            for i, m in enumerate(order):
                for half in range(2):
                    eval_slot(m, half)
                if i == 1:
                    G1 = sb.tile([128, C // 4, C], BF16)
                    G2 = sb.tile([128, C // 4, C], BF16)
                    nc.vector.tensor_add(G1, E_t[:, : C // 4, :],
                                         E_t[:, C // 4: C // 2, :])
                    nc.vector.tensor_add(G2, E_t[:, C // 2: 3 * C // 4, :],
                                         E_t[:, 3 * C // 4:, :])
                    nc.vector.tensor_add(G1, G1, G2)
                if i == 3:
                    nc.vector.tensor_add(G1[:, : C // 8, :], G1[:, : C // 8, :],
                                         G1[:, C // 8: C // 4, :])
                    nc.vector.tensor_add(G1[:, : C // 16, :],
                                         G1[:, : C // 16, :],
                                         G1[:, C // 16: C // 8, :])
                    d_t = sb.tile([128, C], FP32)
                    nc.vector.tensor_reduce(
                        out=d_t, in_=G1[:, : C // 16, :].transpose([0, 2, 1]),
                        axis=mybir.AxisListType.X, op=mybir.AluOpType.add,
                    )
                    r_t = sb.tile([128, C], FP32)
                    nc.vector.reciprocal_approx_fast(out=r_t, in_=d_t)
                    w16 = sb.tile([128, C], BF16)
                    nc.vector.tensor_mul(w16, qE_t, r_t)
                if i == 5:
                    Q4 = C // 4
                    F1 = sb.tile([128, C, Q4], BF16)
                    F2 = sb.tile([128, C, Q4], BF16)
                    F3 = sb.tile([128, C, Q4], BF16)
                    F4 = sb.tile([128, C, Q4], BF16)
                    for fi, Fq in enumerate((F1, F2, F3, F4)):
                        nc.vector.tensor_mul(
                            Fq, E_t[:, :, fi * Q4: (fi + 1) * Q4],
                            w16[:, None, fi * Q4: (fi + 1) * Q4]
                            .broadcast_to([128, C, Q4]),
                        )
                    nc.vector.tensor_add(F1, F1, F2)
                    nc.vector.tensor_add(F3, F3, F4)

            # drain each half's tree; final merge per column-chunk with
            # the output DMA of each chunk issued immediately
            for half in range(2):
                while len(pend[half]) > 2:
                    _, a2 = pend[half].pop()
                    l2, b2 = pend[half].pop()
                    nc.vector.tensor_add(a2, a2, b2)
                    pend[half].append((l2 + 1, a2))
                if len(pend[half]) == 2:
                    _, rootA = pend[half].pop()
                    _, rootB = pend[half].pop()
                    for qd in range(2):
                        ss = slice(qd * 496, (qd + 1) * 496)
                        gs = slice(half * HFD + qd * 496,
                                   half * HFD + (qd + 1) * 496)
                        nc.vector.tensor_add(rootA[:, ss], rootA[:, ss],
                                             rootB[:, ss])
                        nc.sync.dma_start(out=outm[:, gs], in_=rootA[:, ss])
                else:
                    nc.sync.dma_start(
                        out=outm[:, half * HFD:(half + 1) * HFD],
                        in_=pend[half][0][1])


            # exact-tile numerator tail (overlaps the outm DMAs)
            nc.vector.tensor_add(F1, F1, F3)
            Q4 = C // 4
            nc.vector.tensor_add(F1[:, :, : Q4 // 2], F1[:, :, : Q4 // 2],
                                 F1[:, :, Q4 // 2:])
            nc.vector.tensor_add(F1[:, :, : Q4 // 4], F1[:, :, : Q4 // 4],
                                 F1[:, :, Q4 // 4: Q4 // 2])
            oE = sb.tile([128, C], FP32)
            nc.vector.tensor_reduce(
                out=oE, in_=F1[:, :, : Q4 // 4],
                axis=mybir.AxisListType.X, op=mybir.AluOpType.add,
            )
            nc.sync.dma_start(out=oute, in_=oE)

    nc.compile()
    return nc


_NC_CACHE = None


def _get_nc():
    global _NC_CACHE
    if _NC_CACHE is None:
        _NC_CACHE = build_kernel()
    return _NC_CACHE


def _prep(x, y, z):
    """Host prep: sort by difficulty, shard, scale. Returns in_maps + meta."""
    q = np.ascontiguousarray(np.transpose(np.asarray(x), (0, 2, 3, 1))).reshape(-1, C)
    k = np.ascontiguousarray(np.transpose(np.asarray(y), (0, 2, 3, 1))).reshape(-1, C)
    v = np.ascontiguousarray(np.transpose(np.asarray(z), (0, 2, 3, 1))).reshape(-1, C)
    Tk = np.abs(k).max(axis=1)
    A = Tk * np.abs(v).max(axis=1)
    order = np.argsort(A, kind="stable")
    easy = order[: NEZ_CORE * N_CORES]
    hard = order[NEZ_CORE * N_CORES:]

    Lc = _lc_matrix()
    statR = np.zeros((128, M, R2), np.float32)
    for m in range(M):
        for g in range(2):
            statR[g * 64:(g + 1) * 64, m, 2 * m + g] = 1
    statL = np.zeros((128, M, R2), np.float32)
    for r in range(M):
        for m in range(M):
            for g in range(2):
                statL[g * 64:(g + 1) * 64, r, 2 * m + g] = Lc[r, m]
    statB = np.zeros((R2, M, 128), np.float32)
    for m in range(M):
        for g in range(2):
            statB[2 * m + g, m, g * 64:(g + 1) * 64] = 1

    in_maps = []
    meta = []
    for c in range(N_CORES):
        ez = easy[c::N_CORES]
        hd = hard[c::N_CORES]
        kh = k[ez] / Tk[ez, None]
        vp_c = (Tk[ez, None] * v[ez]).astype(np.float16)
        k2_c = (2.0 * kh).astype(np.float16)
        q_c = q[ez].astype(np.float16)
        u1_c = (kh * q[ez]).astype(np.float16)

        def cmaj(a2d, dt):
            h0 = a2d[:FD].T
            h1 = a2d[FD:].T
            return np.ascontiguousarray(np.concatenate([h0, h1], axis=0)).astype(dt)

        in_maps.append({
            "vp": cmaj(vp_c, np.float16),
            "k2t": cmaj(k2_c, np.float16),
            "qt": cmaj(q_c, np.float16),
            "u1t": cmaj(u1_c, np.float16),
            "statR": statR.astype(np.float16),
            "statL": statL.astype(np.float16),
            "statB": statB.astype(np.float16),
            "vE": v[hd].astype(np.float32),
            "kE": k[hd].astype(np.float32),
            "qE": q[hd].astype(np.float32),
        })
        meta.append((ez, hd))
    return in_maps, meta


def kernel(x, y, z):
    nc = _get_nc()
    in_maps, meta = _prep(x, y, z)
    res = run_bass_kernel_spmd(nc, in_maps, core_ids=list(range(N_CORES)))
    out = np.empty((NPIX, C), np.float32)
    for c in range(N_CORES):
        ez, hd = meta[c]
        om = res.results[c]["outm"].astype(np.float32)
        out[ez[:FD]] = om[:64].T
        out[ez[FD:]] = om[64:].T
        out[hd] = res.results[c]["oute"]
    return np.ascontiguousarray(
        np.transpose(out.reshape(B, H, W, C), (0, 3, 1, 2))
    ).astype(np.float32)


# revision 42
# speedup vs baseline: 1.2301x; 1.0037x over previous
"""Trainium2 kernel: per-pixel channel-mixing attention via temperature
interpolation (sigma-interp), v8.

Math per pixel: out_i = sum_j sigma_i(k_j) q_j where sigma(t) = softmax(t*v)
over channels. sigma(t*v) is interpolated in the temperature t at M=11
per-pixel-scaled Chebyshev nodes t_m = Tk*u_m (Tk = max|k| per pixel):

    out_i = sum_m exp(u_m * Tk*v_i) * S_m,   S_m = R_m / G_m
    G_m   = sum_i exp(u_m * Tk*v_i)
    R_m   = sum_r Lc[r,m] * That_r,  That_r = sum_j T_r(k_j/Tk) q_j

M=11 includes the center node u=0 whose grid is identically 1: no exp pass,
no eval multiply; its S broadcast feeds the output sum directly.

Host sorts pixels by A = max|v|*max|k|; the hardest 128 per core go through
an exact pair-grid tile.

Performance structure (v8):
  - R produced directly by Lc-weighted reduction stationaries.
  - chebyshev chain parity-split over z = 2*khat^2-1 (half the serial depth).
  - warmup/filler matmuls run in the unused partition rows of the reduce
    psum tile (same banks, disjoint partitions) to hold the PE clock at
    2.4 GHz through analysis.
  - eval: S broadcast (PE) -> fp16 SBUF copy (ACT) -> 2x DVE multiply ->
    pairwise fp16 DVE add-tree.  No identity-accumulate matmuls at all:
    the eval PE work is just the broadcasts, deeply pipelined through a
    4-buffer PSUM pool (the accumulator banks are freed by the tree).
  - exact-tile work is interleaved into engine slack around the S phase.
  - no Ln activations (reciprocal_approx_fast): one exp table load.
"""

import sys

sys.path.insert(0, "/opt/trn_rl_repo")

from contextlib import ExitStack

import ml_dtypes
import numpy as np

import concourse.bacc as bacc
import concourse.bass as bass
import concourse.tile as tile
from concourse import mybir
from concourse.bass_utils import run_bass_kernel_spmd

B, C, H, W = 2, 64, 128, 128
N_CORES = 8
NPIX = B * H * W            # 32768
M = 11                      # interp nodes (odd: center node u=0 is free)
MC = M // 2
NEX_CORE = 128              # exact pixels per core
NEZ_CORE = NPIX // N_CORES - NEX_CORE   # 3968 interp pixels per core
FD = NEZ_CORE // 2          # 1984 pixels per g-half
HFD = FD // 2               # 992 pixels per column-half
R2 = 2 * M
N_WARM = 24                 # PE warmup matmuls
N_JUNK_ANA = 4              # filler matmuls per analysis m-step
# (half, m) slots whose S stays in PSUM (DVE 1x mul, no ACT copy)
PSUM_MUL_SLOTS = set()

FP32 = mybir.dt.float32
FP16 = mybir.dt.float16
BF16 = mybir.dt.bfloat16
EXP = mybir.ActivationFunctionType.Exp

U_NODES = np.cos(np.pi * np.arange(M) / (M - 1))


def _lc_matrix():
    u = U_NODES
    Tn = np.cos(np.arange(M)[:, None] * np.arccos(np.clip(u, -1, 1))[None, :])
    return np.linalg.inv(Tn.T)


def build_kernel():
    nc = bacc.Bacc(
        "TRN2",
        target_bir_lowering=False,
        debug=False,
        enable_asserts=False,
        num_devices=N_CORES,
    )
    vp = nc.dram_tensor("vp", [128, FD], FP16, kind="ExternalInput").ap()
    k2t = nc.dram_tensor("k2t", [128, FD], FP16, kind="ExternalInput").ap()
    qt = nc.dram_tensor("qt", [128, FD], FP16, kind="ExternalInput").ap()
    u1t = nc.dram_tensor("u1t", [128, FD], FP16, kind="ExternalInput").ap()
    statR = nc.dram_tensor("statR", [128, M, R2], FP16, kind="ExternalInput").ap()
    statL = nc.dram_tensor("statL", [128, M, R2], FP16, kind="ExternalInput").ap()
    statB = nc.dram_tensor("statB", [R2, M, 128], FP16, kind="ExternalInput").ap()
    vE = nc.dram_tensor("vE", [128, C], FP32, kind="ExternalInput").ap()
    kE = nc.dram_tensor("kE", [128, C], FP32, kind="ExternalInput").ap()
    qE = nc.dram_tensor("qE", [128, C], FP32, kind="ExternalInput").ap()

    outm = nc.dram_tensor("outm", [128, FD], FP16, kind="ExternalOutput").ap()
    oute = nc.dram_tensor("oute", [128, C], FP32, kind="ExternalOutput").ap()

    CH = [0, 512, HFD]

    with tile.TileContext(nc) as tc, ExitStack() as ctx:
        sb = ctx.enter_context(tc.tile_pool(name="sb", bufs=1))
        sbw = ctx.enter_context(tc.tile_pool(name="sbw", bufs=1))
        sbp = ctx.enter_context(tc.tile_pool(name="sbp", bufs=2))

        wu = sbw.tile([128, 512], FP16, tag="wu")
        nc.vector.memset(wu, 1.0)

        # dummy activation: pull the exp table load off the critical path
        dum = sbw.tile([1, 16], FP32, tag="dum")
        nc.scalar.activation(out=dum, in_=wu[0:1, 0:16], func=EXP)

        # ---- input DMAs ----
        v_t = sb.tile([128, FD], FP16)
        k2_t = sb.tile([128, FD], FP16)
        q_t = sb.tile([128, FD], FP16)
        u1_t = sb.tile([128, FD], FP16)
        sR = sb.tile([128, M, R2], FP16)
        sL = sb.tile([128, M, R2], FP16)
        sB_t = sb.tile([R2, M, 128], FP16)
        nc.sync.dma_start(out=v_t[:, :FD // 2], in_=vp[:, :FD // 2])
        nc.sync.dma_start(out=k2_t, in_=k2t)
        nc.sync.dma_start(out=u1_t, in_=u1t)
        nc.sync.dma_start(out=q_t, in_=qt)
        nc.sync.dma_start(out=v_t[:, FD // 2:], in_=vp[:, FD // 2:])
        nc.gpsimd.dma_start(out=sR, in_=statR)
        nc.gpsimd.dma_start(out=sL, in_=statL)
        nc.gpsimd.dma_start(out=sB_t, in_=statB)

        ones_t = sbw.tile([128, FD], FP16, tag="ones")
        nc.vector.memset(ones_t, 1.0)

        # exact-tile small inputs (early: cheap, off the critical path)
        vE_t = sb.tile([128, C], FP32)
        nc.sync.dma_start(out=vE_t, in_=vE)
        kE_t = sb.tile([128, C], FP32)
        nc.sync.dma_start(out=kE_t, in_=kE)
        qE_t = sb.tile([128, C], FP32)
        nc.sync.dma_start(out=qE_t, in_=qE)
        v2 = sb.tile([128, C, 2], FP16)
        nc.scalar.copy(v2, vE_t[:, :, None].broadcast_to([128, C, 2]))
        kE16 = sb.tile([128, C], FP16)
        nc.scalar.copy(kE16, kE_t)

        # ---- X grids (ACT, fp16), per column-half for early start ----
        Xs = {}
        for m in range(M):
            if m == MC:
                Xs[m] = ones_t
                continue
            xm = sbw.tile([128, FD], FP16, tag=f"x{m}")
            nc.scalar.activation(out=xm[:, :FD // 2], in_=v_t[:, :FD // 2],
                                 func=EXP, scale=float(U_NODES[m]))
            nc.scalar.activation(out=xm[:, FD // 2:], in_=v_t[:, FD // 2:],
                                 func=EXP, scale=float(U_NODES[m]))
            Xs[m] = xm

        # ---- chebyshev chain, parity split over zz = 2z = k2^2 - 2 ----
        zz = sbw.tile([128, FD], FP16, tag="zz")
        nc.vector.tensor_mul(zz, k2_t, k2_t)
        nc.vector.tensor_scalar_add(zz, zz, -2.0)
        Us = {0: q_t, 1: u1_t}
        e1 = sbw.tile([128, FD], FP16, tag="e1")
        nc.vector.scalar_tensor_tensor(
            out=e1, in0=zz, scalar=0.5, in1=q_t,
            op0=mybir.AluOpType.mult, op1=mybir.AluOpType.mult)
        Us[2] = e1
        o1 = sbw.tile([128, FD], FP16, tag="o1")
        nc.vector.scalar_tensor_tensor(
            out=o1, in0=zz, scalar=-1.0, in1=u1_t,
            op0=mybir.AluOpType.add, op1=mybir.AluOpType.mult)
        Us[3] = o1
        for par in (0, 1):
            prev, cur = Us[0 + par], Us[2 + par]
            for r in range(4 + par, M, 2):
                tmp = sbp.tile([128, FD], FP16, tag=f"tmp{par}")
                nc.vector.tensor_mul(tmp, zz, cur)
                nxt = sbw.tile([128, FD], FP16, tag=f"u{r}")
                nc.vector.tensor_sub(nxt, tmp, prev)
                Us[r] = nxt
                prev, cur = cur, nxt

        # exact pair grid (DVE work early; ACT exp deferred to S phase)
        P_t = sb.tile([128, C, C], FP16)
        k_op = bass.AP(
            tensor=kE16.tensor, offset=kE16.offset,
            ap=[kE16.ap[0], [0, C], [2, C // 2], [1, 2]],
        )
        v_op = bass.AP(
            tensor=v2.tensor, offset=v2.offset,
            ap=[v2.ap[0], [2, C], [0, C // 2], [1, 2]],
        )
        nc.vector.tensor_mul(
            P_t.rearrange("p i (jh jp) -> p i jh jp", jp=2), k_op, v_op)

        # ---- analysis reductions: R rows 0:22 (Lc-weighted), G rows 32:54
        # warmup + filler matmuls live in rows 64:128 of the same tiles.
        with tc.tile_pool(name="red", bufs=2, space="PSUM") as redp:
            red_ts = [redp.tile([128, HFD], FP32, tag="red", name=f"red{h}")
                      for h in range(2)]
            _junk_i = [0]

            def junk_mm(n):
                for _ in range(n):
                    t = red_ts[_junk_i[0] % 2]
                    _junk_i[0] += 1
                    nc.tensor.matmul(t[64:128, 0:496], wu[:, 0:64],
                                     wu[:, 0:496], start=True, stop=True,
                                     skip_group_check=True)

            junk_mm(N_WARM)
            for m in range(M):
                for h in range(2):
                    hs = slice(h * HFD, (h + 1) * HFD)
                    r_ps = red_ts[h][0:R2, :]
                    g_ps = red_ts[h][32:32 + R2, :]
                    for a, b in zip(CH[:-1], CH[1:]):
                        nc.tensor.matmul(r_ps[:, a:b], sL[:, m, :],
                                         Us[m][:, hs][:, a:b],
                                         start=(m == 0), stop=(m == M - 1))
                        nc.tensor.matmul(g_ps[:, a:b], sR[:, m, :],
                                         Xs[m][:, hs][:, a:b],
                                         start=(m == 0), stop=(m == M - 1))
                junk_mm(N_JUNK_ANA)

            # ---- S = R / G per half (highest scheduler priority: the
            # whole eval stream is gated on these few small ops) ----
            s_halves = []
            for h in range(2):
                r_ps = red_ts[h][0:R2, :]
                g_ps = red_ts[h][32:32 + R2, :]
                gsb = sbw.tile([R2, HFD], FP32, tag=f"gsb{h}")
                nc.scalar.copy(gsb, g_ps)
                junk_mm(2)
                ginv = sbw.tile([R2, HFD], FP32, tag=f"ginv{h}")
                nc.vector.reciprocal_approx_fast(out=ginv, in_=gsb)
                s_th = sbw.tile([R2, HFD], FP16, tag=f"s{h}")
                nc.vector.tensor_mul(s_th, ginv, r_ps)
                s_halves.append(s_th)
                junk_mm(1)

        # exact-tile exp: scalar-engine slack right after the X stream
        E_t = sb.tile([128, C, C], BF16)
        for eb in range(4):
            nc.scalar.activation(out=E_t[:, eb * 16:(eb + 1) * 16, :],
                                 in_=P_t[:, eb * 16:(eb + 1) * 16, :],
                                 func=EXP)

        # ---- eval: out = sum_m X_m * bcast(S_m), fp16 DVE add-tree ----
        with tc.tile_pool(name="evp", bufs=4, space="PSUM") as evp:
            pend = {0: [], 1: []}   # binary-counter tree: (level, tile)

            def tree_push(half, t, level=0):
                pend[half].append((level, t))
                while (len(pend[half]) >= 2
                       and pend[half][-1][0] == pend[half][-2][0]):
                    l1, a2 = pend[half].pop()
                    _, b2 = pend[half].pop()
                    nc.vector.tensor_add(a2, a2, b2)
                    pend[half].append((l1 + 1, a2))

            def eval_slot(m, half):
                s_h = s_halves[half]
                s_b = evp.tile([128, HFD], FP32, tag="sbps",
                               name=f"sb{half}_{m}")
                for a, b in zip(CH[:-1], CH[1:]):
                    nc.tensor.matmul(s_b[:, a:b], sB_t[:, m, :],
                                     s_h[:, a:b], start=True, stop=True)
                if m == MC:
                    s_bs = sbp.tile([128, HFD], FP16, tag="sbs", bufs=6)
                    nc.scalar.copy(s_bs, s_b)
                    tree_push(half, s_bs)
                    return
                prod = sbp.tile([128, HFD], FP16, tag="prod", bufs=8)
                if (half, m) in PSUM_MUL_SLOTS:
                    nc.vector.tensor_mul(
                        prod, Xs[m][:, half * HFD:(half + 1) * HFD], s_b)
                else:
                    s_bs = sbp.tile([128, HFD], FP16, tag="sbs", bufs=6)
                    nc.scalar.copy(s_bs, s_b)
                    nc.vector.tensor_mul(
                        prod, Xs[m][:, half * HFD:(half + 1) * HFD], s_bs)
                tree_push(half, prod)

            # order: MC first (cheap), then the rest; halves interleaved
            order = [MC] + [m for m in range(M) if m != MC]
            for i, m in enumerate(order):
                for half in range(2):
                    eval_slot(m, half)
                if i == 1:
                    G1 = sb.tile([128, C // 4, C], BF16)
                    G2 = sb.tile([128, C // 4, C], BF16)
                    nc.vector.tensor_add(G1, E_t[:, : C // 4, :],
                                         E_t[:, C // 4: C // 2, :])
                    nc.vector.tensor_add(G2, E_t[:, C // 2: 3 * C // 4, :],
                                         E_t[:, 3 * C // 4:, :])
                    nc.vector.tensor_add(G1, G1, G2)
                if i == 3:
                    nc.vector.tensor_add(G1[:, : C // 8, :], G1[:, : C // 8, :],
                                         G1[:, C // 8: C // 4, :])
                    nc.vector.tensor_add(G1[:, : C // 16, :],
                                         G1[:, : C // 16, :],
                                         G1[:, C // 16: C // 8, :])
                    d_t = sb.tile([128, C], FP32)
                    nc.vector.tensor_reduce(
                        out=d_t, in_=G1[:, : C // 16, :].transpose([0, 2, 1]),
                        axis=mybir.AxisListType.X, op=mybir.AluOpType.add,
                    )
                    r_t = sb.tile([128, C], FP32)
                    nc.vector.reciprocal_approx_fast(out=r_t, in_=d_t)
                    w16 = sb.tile([128, C], BF16)
                    nc.vector.tensor_mul(w16, qE_t, r_t)
                if i == 5:
                    Q4 = C // 4
                    F1 = sb.tile([128, C, Q4], BF16)
                    F2 = sb.tile([128, C, Q4], BF16)
                    F3 = sb.tile([128, C, Q4], BF16)
                    F4 = sb.tile([128, C, Q4], BF16)
                    for fi, Fq in enumerate((F1, F2, F3, F4)):
                        nc.vector.tensor_mul(
                            Fq, E_t[:, :, fi * Q4: (fi + 1) * Q4],
                            w16[:, None, fi * Q4: (fi + 1) * Q4]
                            .broadcast_to([128, C, Q4]),
                        )
                    nc.vector.tensor_add(F1, F1, F2)
                    nc.vector.tensor_add(F3, F3, F4)

            # drain each half's tree; final merge per column-chunk with
            # the output DMA of each chunk issued immediately
            for half in range(2):
                while len(pend[half]) > 2:
                    _, a2 = pend[half].pop()
                    l2, b2 = pend[half].pop()
                    nc.vector.tensor_add(a2, a2, b2)
                    pend[half].append((l2 + 1, a2))
                if len(pend[half]) == 2:
                    _, rootA = pend[half].pop()
                    _, rootB = pend[half].pop()
                    for qd in range(2):
                        ss = slice(qd * 496, (qd + 1) * 496)
                        gs = slice(half * HFD + qd * 496,
                                   half * HFD + (qd + 1) * 496)
                        nc.vector.tensor_add(rootA[:, ss], rootA[:, ss],
                                             rootB[:, ss])
                        nc.sync.dma_start(out=outm[:, gs], in_=rootA[:, ss])
                else:
                    nc.sync.dma_start(
                        out=outm[:, half * HFD:(half + 1) * HFD],
                        in_=pend[half][0][1])


            # exact-tile numerator tail (overlaps the outm DMAs)
            nc.vector.tensor_add(F1, F1, F3)
            Q4 = C // 4
            nc.vector.tensor_add(F1[:, :, : Q4 // 2], F1[:, :, : Q4 // 2],
                                 F1[:, :, Q4 // 2:])
            nc.vector.tensor_add(F1[:, :, : Q4 // 4], F1[:, :, : Q4 // 4],
                                 F1[:, :, Q4 // 4: Q4 // 2])
            oE = sb.tile([128, C], FP32)
            nc.vector.tensor_reduce(
                out=oE, in_=F1[:, :, : Q4 // 4],
                axis=mybir.AxisListType.X, op=mybir.AluOpType.add,
            )
            nc.sync.dma_start(out=oute, in_=oE)

    nc.compile()
    return nc


_NC_CACHE = None


def _get_nc():
    global _NC_CACHE
    if _NC_CACHE is None:
        _NC_CACHE = build_kernel()
    return _NC_CACHE


def _prep(x, y, z):
    """Host prep: sort by difficulty, shard, scale. Returns in_maps + meta."""
    q = np.ascontiguousarray(np.transpose(np.asarray(x), (0, 2, 3, 1))).reshape(-1, C)
    k = np.ascontiguousarray(np.transpose(np.asarray(y), (0, 2, 3, 1))).reshape(-1, C)
    v = np.ascontiguousarray(np.transpose(np.asarray(z), (0, 2, 3, 1))).reshape(-1, C)
    Tk = np.abs(k).max(axis=1)
    A = Tk * np.abs(v).max(axis=1)
    order = np.argsort(A, kind="stable")
    easy = order[: NEZ_CORE * N_CORES]
    hard = order[NEZ_CORE * N_CORES:]

    Lc = _lc_matrix()
    statR = np.zeros((128, M, R2), np.float32)
    for m in range(M):
        for g in range(2):
            statR[g * 64:(g + 1) * 64, m, 2 * m + g] = 1
    statL = np.zeros((128, M, R2), np.float32)
    for r in range(M):
        for m in range(M):
            for g in range(2):
                statL[g * 64:(g + 1) * 64, r, 2 * m + g] = Lc[r, m]
    statB = np.zeros((R2, M, 128), np.float32)
    for m in range(M):
        for g in range(2):
            statB[2 * m + g, m, g * 64:(g + 1) * 64] = 1

    in_maps = []
    meta = []
    for c in range(N_CORES):
        ez = easy[c::N_CORES]
        hd = hard[c::N_CORES]
        kh = k[ez] / Tk[ez, None]
        vp_c = (Tk[ez, None] * v[ez]).astype(np.float16)
        k2_c = (2.0 * kh).astype(np.float16)
        q_c = q[ez].astype(np.float16)
        u1_c = (kh * q[ez]).astype(np.float16)

        def cmaj(a2d, dt):
            h0 = a2d[:FD].T
            h1 = a2d[FD:].T
            return np.ascontiguousarray(np.concatenate([h0, h1], axis=0)).astype(dt)

        in_maps.append({
            "vp": cmaj(vp_c, np.float16),
            "k2t": cmaj(k2_c, np.float16),
            "qt": cmaj(q_c, np.float16),
            "u1t": cmaj(u1_c, np.float16),
            "statR": statR.astype(np.float16),
            "statL": statL.astype(np.float16),
            "statB": statB.astype(np.float16),
            "vE": v[hd].astype(np.float32),
            "kE": k[hd].astype(np.float32),
            "qE": q[hd].astype(np.float32),
        })
        meta.append((ez, hd))
    return in_maps, meta


def kernel(x, y, z):
    nc = _get_nc()
    in_maps, meta = _prep(x, y, z)
    res = run_bass_kernel_spmd(nc, in_maps, core_ids=list(range(N_CORES)))
    out = np.empty((NPIX, C), np.float32)
    for c in range(N_CORES):
        ez, hd = meta[c]
        om = res.results[c]["outm"].astype(np.float32)
        out[ez[:FD]] = om[:64].T
        out[ez[FD:]] = om[64:].T
        out[hd] = res.results[c]["oute"]
    return np.ascontiguousarray(
        np.transpose(out.reshape(B, H, W, C), (0, 3, 1, 2))
    ).astype(np.float32)


# revision 43
# speedup vs baseline: 1.2601x; 1.0244x over previous
"""Trainium2 kernel: per-pixel channel-mixing attention via temperature
interpolation (sigma-interp), v8.

Math per pixel: out_i = sum_j sigma_i(k_j) q_j where sigma(t) = softmax(t*v)
over channels. sigma(t*v) is interpolated in the temperature t at M=11
per-pixel-scaled Chebyshev nodes t_m = Tk*u_m (Tk = max|k| per pixel):

    out_i = sum_m exp(u_m * Tk*v_i) * S_m,   S_m = R_m / G_m
    G_m   = sum_i exp(u_m * Tk*v_i)
    R_m   = sum_r Lc[r,m] * That_r,  That_r = sum_j T_r(k_j/Tk) q_j

M=11 includes the center node u=0 whose grid is identically 1: no exp pass,
no eval multiply; its S broadcast feeds the output sum directly.

Host sorts pixels by A = max|v|*max|k|; the hardest 128 per core go through
an exact pair-grid tile.

Performance structure (v8):
  - R produced directly by Lc-weighted reduction stationaries.
  - chebyshev chain parity-split over z = 2*khat^2-1 (half the serial depth).
  - warmup/filler matmuls run in the unused partition rows of the reduce
    psum tile (same banks, disjoint partitions) to hold the PE clock at
    2.4 GHz through analysis.
  - eval: S broadcast (PE) -> fp16 SBUF copy (ACT) -> 2x DVE multiply ->
    pairwise fp16 DVE add-tree.  No identity-accumulate matmuls at all:
    the eval PE work is just the broadcasts, deeply pipelined through a
    4-buffer PSUM pool (the accumulator banks are freed by the tree).
  - exact-tile work is interleaved into engine slack around the S phase.
  - no Ln activations (reciprocal_approx_fast): one exp table load.
"""

import sys

sys.path.insert(0, "/opt/trn_rl_repo")

from contextlib import ExitStack

import ml_dtypes
import numpy as np

import concourse.bacc as bacc
import concourse.bass as bass
import concourse.tile as tile
from concourse import mybir
from concourse.bass_utils import run_bass_kernel_spmd

B, C, H, W = 2, 64, 128, 128
N_CORES = 8
NPIX = B * H * W            # 32768
M = 11                      # interp nodes (odd: center node u=0 is free)
MC = M // 2
NEX_CORE = 128              # exact pixels per core
NEZ_CORE = NPIX // N_CORES - NEX_CORE   # 3968 interp pixels per core
FD = NEZ_CORE // 2          # 1984 pixels per g-half
HFD = FD // 2               # 992 pixels per column-half
R2 = 2 * M
N_WARM = 24                 # PE warmup matmuls
N_JUNK_ANA = 4              # filler matmuls per analysis m-step
# (half, m) slots whose S stays in PSUM (DVE 1x mul, no ACT copy)
PSUM_MUL_SLOTS = set()

FP32 = mybir.dt.float32
FP16 = mybir.dt.float16
BF16 = mybir.dt.bfloat16
EXP = mybir.ActivationFunctionType.Exp

U_NODES = np.cos(np.pi * np.arange(M) / (M - 1))


def _lc_matrix():
    u = U_NODES
    Tn = np.cos(np.arange(M)[:, None] * np.arccos(np.clip(u, -1, 1))[None, :])
    return np.linalg.inv(Tn.T)


def build_kernel():
    nc = bacc.Bacc(
        "TRN2",
        target_bir_lowering=False,
        debug=False,
        enable_asserts=False,
        num_devices=N_CORES,
    )
    vp = nc.dram_tensor("vp", [128, FD], FP16, kind="ExternalInput").ap()
    k2t = nc.dram_tensor("k2t", [128, FD], FP16, kind="ExternalInput").ap()
    qt = nc.dram_tensor("qt", [128, FD], FP16, kind="ExternalInput").ap()
    u1t = nc.dram_tensor("u1t", [128, FD], FP16, kind="ExternalInput").ap()
    statR = nc.dram_tensor("statR", [128, M, R2], FP16, kind="ExternalInput").ap()
    statL = nc.dram_tensor("statL", [128, M, R2], FP16, kind="ExternalInput").ap()
    statB = nc.dram_tensor("statB", [R2, M, 128], FP16, kind="ExternalInput").ap()
    vE = nc.dram_tensor("vE", [128, C], FP32, kind="ExternalInput").ap()
    kE = nc.dram_tensor("kE", [128, C], FP32, kind="ExternalInput").ap()
    qE = nc.dram_tensor("qE", [128, C], FP32, kind="ExternalInput").ap()

    outm = nc.dram_tensor("outm", [128, FD], FP16, kind="ExternalOutput").ap()
    oute = nc.dram_tensor("oute", [128, C], FP32, kind="ExternalOutput").ap()

    CH = [0, 512, HFD]

    with tile.TileContext(nc) as tc, ExitStack() as ctx:
        sb = ctx.enter_context(tc.tile_pool(name="sb", bufs=1))
        sbw = ctx.enter_context(tc.tile_pool(name="sbw", bufs=1))
        sbp = ctx.enter_context(tc.tile_pool(name="sbp", bufs=2))

        wu = sbw.tile([128, 512], FP16, tag="wu")
        nc.vector.memset(wu, 1.0)

        # dummy activation: pull the exp table load off the critical path
        dum = sbw.tile([1, 16], FP32, tag="dum")
        nc.scalar.activation(out=dum, in_=wu[0:1, 0:16], func=EXP)

        # ---- input DMAs ----
        v_t = sb.tile([128, FD], FP16)
        k2_t = sb.tile([128, FD], FP16)
        q_t = sb.tile([128, FD], FP16)
        u1_t = sb.tile([128, FD], FP16)
        sR = sb.tile([128, M, R2], FP16)
        sL = sb.tile([128, M, R2], FP16)
        sB_t = sb.tile([R2, M, 128], FP16)
        nc.sync.dma_start(out=v_t[:, :FD // 2], in_=vp[:, :FD // 2])
        nc.sync.dma_start(out=k2_t, in_=k2t)
        nc.sync.dma_start(out=u1_t, in_=u1t)
        nc.sync.dma_start(out=q_t, in_=qt)
        nc.sync.dma_start(out=v_t[:, FD // 2:], in_=vp[:, FD // 2:])
        nc.gpsimd.dma_start(out=sR, in_=statR)
        nc.gpsimd.dma_start(out=sL, in_=statL)
        nc.gpsimd.dma_start(out=sB_t, in_=statB)

        ones_t = sbw.tile([128, FD], FP16, tag="ones")
        nc.vector.memset(ones_t, 1.0)

        # exact-tile small inputs (early: cheap, off the critical path)
        vE_t = sb.tile([128, C], FP32)
        nc.sync.dma_start(out=vE_t, in_=vE)
        kE_t = sb.tile([128, C], FP32)
        nc.sync.dma_start(out=kE_t, in_=kE)
        qE_t = sb.tile([128, C], FP32)
        nc.sync.dma_start(out=qE_t, in_=qE)
        v2 = sb.tile([128, C, 2], FP16)
        nc.scalar.copy(v2, vE_t[:, :, None].broadcast_to([128, C, 2]))
        kE16 = sb.tile([128, C], FP16)
        nc.scalar.copy(kE16, kE_t)

        # ---- X grids (ACT, fp16), per column-half for early start ----
        Xs = {}
        for m in range(M):
            if m == MC:
                Xs[m] = ones_t
                continue
            xm = sbw.tile([128, FD], FP16, tag=f"x{m}")
            nc.scalar.activation(out=xm[:, :FD // 2], in_=v_t[:, :FD // 2],
                                 func=EXP, scale=float(U_NODES[m]))
            nc.scalar.activation(out=xm[:, FD // 2:], in_=v_t[:, FD // 2:],
                                 func=EXP, scale=float(U_NODES[m]))
            Xs[m] = xm

        # ---- chebyshev chain, parity split over zz = 2z = k2^2 - 2 ----
        zz = sbw.tile([128, FD], FP16, tag="zz")
        nc.vector.tensor_mul(zz, k2_t, k2_t)
        nc.vector.tensor_scalar_add(zz, zz, -2.0)
        Us = {0: q_t, 1: u1_t}
        e1 = sbw.tile([128, FD], FP16, tag="e1")
        nc.vector.scalar_tensor_tensor(
            out=e1, in0=zz, scalar=0.5, in1=q_t,
            op0=mybir.AluOpType.mult, op1=mybir.AluOpType.mult)
        Us[2] = e1
        o1 = sbw.tile([128, FD], FP16, tag="o1")
        nc.vector.scalar_tensor_tensor(
            out=o1, in0=zz, scalar=-1.0, in1=u1_t,
            op0=mybir.AluOpType.add, op1=mybir.AluOpType.mult)
        Us[3] = o1
        for par in (0, 1):
            prev, cur = Us[0 + par], Us[2 + par]
            for r in range(4 + par, M, 2):
                tmp = sbp.tile([128, FD], FP16, tag=f"tmp{par}")
                nc.vector.tensor_mul(tmp, zz, cur)
                nxt = sbw.tile([128, FD], FP16, tag=f"u{r}")
                nc.vector.tensor_sub(nxt, tmp, prev)
                Us[r] = nxt
                prev, cur = cur, nxt

        # exact pair grid (DVE work early; ACT exp deferred to S phase)
        P_t = sb.tile([128, C, C], FP16)
        k_op = bass.AP(
            tensor=kE16.tensor, offset=kE16.offset,
            ap=[kE16.ap[0], [0, C], [2, C // 2], [1, 2]],
        )
        v_op = bass.AP(
            tensor=v2.tensor, offset=v2.offset,
            ap=[v2.ap[0], [2, C], [0, C // 2], [1, 2]],
        )
        nc.vector.tensor_mul(
            P_t.rearrange("p i (jh jp) -> p i jh jp", jp=2), k_op, v_op)

        # ---- analysis reductions: R rows 0:22 (Lc-weighted), G rows 32:54
        # warmup + filler matmuls live in rows 64:128 of the same tiles.
        with tc.tile_pool(name="red", bufs=2, space="PSUM") as redp:
            red_ts = [redp.tile([128, HFD], FP32, tag="red", name=f"red{h}")
                      for h in range(2)]
            _junk_i = [0]

            def junk_mm(n):
                for _ in range(n):
                    t = red_ts[_junk_i[0] % 2]
                    _junk_i[0] += 1
                    nc.tensor.matmul(t[64:128, 0:496], wu[:, 0:64],
                                     wu[:, 0:496], start=True, stop=True,
                                     skip_group_check=True)

            junk_mm(N_WARM)
            for m in range(M):
                for h in range(2):
                    hs = slice(h * HFD, (h + 1) * HFD)
                    r_ps = red_ts[h][0:R2, :]
                    g_ps = red_ts[h][32:32 + R2, :]
                    for a, b in zip(CH[:-1], CH[1:]):
                        nc.tensor.matmul(r_ps[:, a:b], sL[:, m, :],
                                         Us[m][:, hs][:, a:b],
                                         start=(m == 0), stop=(m == M - 1))
                        nc.tensor.matmul(g_ps[:, a:b], sR[:, m, :],
                                         Xs[m][:, hs][:, a:b],
                                         start=(m == 0), stop=(m == M - 1))
                junk_mm(N_JUNK_ANA)

            # ---- S = R / G per half (highest scheduler priority: the
            # whole eval stream is gated on these few small ops) ----
            s_halves = []
            for h in range(2):
                r_ps = red_ts[h][0:R2, :]
                g_ps = red_ts[h][32:32 + R2, :]
                gsb = sbw.tile([R2, HFD], FP32, tag=f"gsb{h}")
                nc.scalar.copy(gsb, g_ps)
                junk_mm(2)
                ginv = sbw.tile([R2, HFD], FP32, tag=f"ginv{h}")
                nc.vector.reciprocal_approx_fast(out=ginv, in_=gsb)
                s_th = sbw.tile([R2, HFD], FP16, tag=f"s{h}")
                nc.vector.tensor_mul(s_th, ginv, r_ps)
                s_halves.append(s_th)
                junk_mm(1)

        # exact-tile exp: scalar-engine slack right after the X stream
        E_t = sb.tile([128, C, C], BF16)
        for eb in range(4):
            nc.scalar.activation(out=E_t[:, eb * 16:(eb + 1) * 16, :],
                                 in_=P_t[:, eb * 16:(eb + 1) * 16, :],
                                 func=EXP)

        # ---- eval: out = sum_m X_m * bcast(S_m), fp16 DVE add-tree ----
        with tc.tile_pool(name="evp", bufs=4, space="PSUM") as evp:
            pend = {0: [], 1: []}   # binary-counter tree: (level, tile)

            def tree_push(half, t, level=0):
                pend[half].append((level, t))
                while (len(pend[half]) >= 2
                       and pend[half][-1][0] == pend[half][-2][0]):
                    l1, a2 = pend[half].pop()
                    _, b2 = pend[half].pop()
                    nc.vector.tensor_add(a2, a2, b2)
                    pend[half].append((l1 + 1, a2))

            def eval_slot(m, half):
                s_h = s_halves[half]
                s_b = evp.tile([128, HFD], FP32, tag="sbps",
                               name=f"sb{half}_{m}")
                for a, b in zip(CH[:-1], CH[1:]):
                    nc.tensor.matmul(s_b[:, a:b], sB_t[:, m, :],
                                     s_h[:, a:b], start=True, stop=True)
                if m == MC:
                    s_bs = sbp.tile([128, HFD], FP16, tag="sbs", bufs=8)
                    nc.scalar.copy(s_bs, s_b)
                    tree_push(half, s_bs)
                    return
                prod = sbp.tile([128, HFD], FP16, tag="prod", bufs=10)
                if (half, m) in PSUM_MUL_SLOTS:
                    nc.vector.tensor_mul(
                        prod, Xs[m][:, half * HFD:(half + 1) * HFD], s_b)
                else:
                    s_bs = sbp.tile([128, HFD], FP16, tag="sbs", bufs=8)
                    nc.scalar.copy(s_bs, s_b)
                    nc.vector.tensor_mul(
                        prod, Xs[m][:, half * HFD:(half + 1) * HFD], s_bs)
                tree_push(half, prod)

            # order: MC first (cheap), then the rest; halves interleaved
            order = [MC] + [m for m in range(M) if m != MC]
            for i, m in enumerate(order):
                for half in range(2):
                    eval_slot(m, half)
                if i == 1:
                    G1 = sb.tile([128, C // 4, C], BF16)
                    G2 = sb.tile([128, C // 4, C], BF16)
                    nc.vector.tensor_add(G1, E_t[:, : C // 4, :],
                                         E_t[:, C // 4: C // 2, :])
                    nc.vector.tensor_add(G2, E_t[:, C // 2: 3 * C // 4, :],
                                         E_t[:, 3 * C // 4:, :])
                    nc.vector.tensor_add(G1, G1, G2)
                if i == 3:
                    nc.vector.tensor_add(G1[:, : C // 8, :], G1[:, : C // 8, :],
                                         G1[:, C // 8: C // 4, :])
                    nc.vector.tensor_add(G1[:, : C // 16, :],
                                         G1[:, : C // 16, :],
                                         G1[:, C // 16: C // 8, :])
                    d_t = sb.tile([128, C], FP32)
                    nc.vector.tensor_reduce(
                        out=d_t, in_=G1[:, : C // 16, :].transpose([0, 2, 1]),
                        axis=mybir.AxisListType.X, op=mybir.AluOpType.add,
                    )
                    r_t = sb.tile([128, C], FP32)
                    nc.vector.reciprocal_approx_fast(out=r_t, in_=d_t)
                    w16 = sb.tile([128, C], BF16)
                    nc.vector.tensor_mul(w16, qE_t, r_t)
                if i == 5:
                    Q4 = C // 4
                    F1 = sb.tile([128, C, Q4], BF16)
                    F2 = sb.tile([128, C, Q4], BF16)
                    F3 = sb.tile([128, C, Q4], BF16)
                    F4 = sb.tile([128, C, Q4], BF16)
                    for fi, Fq in enumerate((F1, F2, F3, F4)):
                        nc.vector.tensor_mul(
                            Fq, E_t[:, :, fi * Q4: (fi + 1) * Q4],
                            w16[:, None, fi * Q4: (fi + 1) * Q4]
                            .broadcast_to([128, C, Q4]),
                        )
                    nc.vector.tensor_add(F1, F1, F2)
                    nc.vector.tensor_add(F3, F3, F4)

            # drain each half's tree; final merge per column-chunk with
            # the output DMA of each chunk issued immediately
            for half in range(2):
                while len(pend[half]) > 2:
                    _, a2 = pend[half].pop()
                    l2, b2 = pend[half].pop()
                    nc.vector.tensor_add(a2, a2, b2)
                    pend[half].append((l2 + 1, a2))
                if len(pend[half]) == 2:
                    _, rootA = pend[half].pop()
                    _, rootB = pend[half].pop()
                    for qd in range(2):
                        ss = slice(qd * 496, (qd + 1) * 496)
                        gs = slice(half * HFD + qd * 496,
                                   half * HFD + (qd + 1) * 496)
                        nc.vector.tensor_add(rootA[:, ss], rootA[:, ss],
                                             rootB[:, ss])
                        nc.sync.dma_start(out=outm[:, gs], in_=rootA[:, ss])
                else:
                    nc.sync.dma_start(
                        out=outm[:, half * HFD:(half + 1) * HFD],
                        in_=pend[half][0][1])


            # exact-tile numerator tail (overlaps the outm DMAs)
            nc.vector.tensor_add(F1, F1, F3)
            Q4 = C // 4
            nc.vector.tensor_add(F1[:, :, : Q4 // 2], F1[:, :, : Q4 // 2],
                                 F1[:, :, Q4 // 2:])
            nc.vector.tensor_add(F1[:, :, : Q4 // 4], F1[:, :, : Q4 // 4],
                                 F1[:, :, Q4 // 4: Q4 // 2])
            oE = sb.tile([128, C], FP32)
            nc.vector.tensor_reduce(
                out=oE, in_=F1[:, :, : Q4 // 4],
                axis=mybir.AxisListType.X, op=mybir.AluOpType.add,
            )
            nc.sync.dma_start(out=oute, in_=oE)

    nc.compile()
    return nc


_NC_CACHE = None


def _get_nc():
    global _NC_CACHE
    if _NC_CACHE is None:
        _NC_CACHE = build_kernel()
    return _NC_CACHE


def _prep(x, y, z):
    """Host prep: sort by difficulty, shard, scale. Returns in_maps + meta."""
    q = np.ascontiguousarray(np.transpose(np.asarray(x), (0, 2, 3, 1))).reshape(-1, C)
    k = np.ascontiguousarray(np.transpose(np.asarray(y), (0, 2, 3, 1))).reshape(-1, C)
    v = np.ascontiguousarray(np.transpose(np.asarray(z), (0, 2, 3, 1))).reshape(-1, C)
    Tk = np.abs(k).max(axis=1)
    A = Tk * np.abs(v).max(axis=1)
    order = np.argsort(A, kind="stable")
    easy = order[: NEZ_CORE * N_CORES]
    hard = order[NEZ_CORE * N_CORES:]

    Lc = _lc_matrix()
    statR = np.zeros((128, M, R2), np.float32)
    for m in range(M):
        for g in range(2):
            statR[g * 64:(g + 1) * 64, m, 2 * m + g] = 1
    statL = np.zeros((128, M, R2), np.float32)
    for r in range(M):
        for m in range(M):
            for g in range(2):
                statL[g * 64:(g + 1) * 64, r, 2 * m + g] = Lc[r, m]
    statB = np.zeros((R2, M, 128), np.float32)
    for m in range(M):
        for g in range(2):
            statB[2 * m + g, m, g * 64:(g + 1) * 64] = 1

    in_maps = []
    meta = []
    for c in range(N_CORES):
        ez = easy[c::N_CORES]
        hd = hard[c::N_CORES]
        kh = k[ez] / Tk[ez, None]
        vp_c = (Tk[ez, None] * v[ez]).astype(np.float16)
        k2_c = (2.0 * kh).astype(np.float16)
        q_c = q[ez].astype(np.float16)
        u1_c = (kh * q[ez]).astype(np.float16)

        def cmaj(a2d, dt):
            h0 = a2d[:FD].T
            h1 = a2d[FD:].T
            return np.ascontiguousarray(np.concatenate([h0, h1], axis=0)).astype(dt)

        in_maps.append({
            "vp": cmaj(vp_c, np.float16),
            "k2t": cmaj(k2_c, np.float16),
            "qt": cmaj(q_c, np.float16),
            "u1t": cmaj(u1_c, np.float16),
            "statR": statR.astype(np.float16),
            "statL": statL.astype(np.float16),
            "statB": statB.astype(np.float16),
            "vE": v[hd].astype(np.float32),
            "kE": k[hd].astype(np.float32),
            "qE": q[hd].astype(np.float32),
        })
        meta.append((ez, hd))
    return in_maps, meta


def kernel(x, y, z):
    nc = _get_nc()
    in_maps, meta = _prep(x, y, z)
    res = run_bass_kernel_spmd(nc, in_maps, core_ids=list(range(N_CORES)))
    out = np.empty((NPIX, C), np.float32)
    for c in range(N_CORES):
        ez, hd = meta[c]
        om = res.results[c]["outm"].astype(np.float32)
        out[ez[:FD]] = om[:64].T
        out[ez[FD:]] = om[64:].T
        out[hd] = res.results[c]["oute"]
    return np.ascontiguousarray(
        np.transpose(out.reshape(B, H, W, C), (0, 3, 1, 2))
    ).astype(np.float32)
